# revision 1
# baseline (speedup 1.0000x reference)
"""Trainium2 Bass kernel for nn_Block_32762010534337 (dense transformer block).

Strategy: sequence-parallel over 8 cores. Core c owns 512 tokens (batch c//4,
token chunk c%4). Each core computes rmsnorm -> K/V projections (+rope, k-norm)
for its own tokens, AllGathers K/V within its batch group of 4 cores (overlapped
with the Q projections), then runs causal attention + wo + MLP (relu^2) for its
512 tokens with fully replicated bf16 weights. Activations stay feature-major
([channel, token]); the host transposes per-core inputs and the final residual
path switches to token-major via 64 PE transposes.
"""
import sys
import os

if "/opt/trn_rl_repo" not in sys.path:
    sys.path.insert(0, "/opt/trn_rl_repo")

import numpy as np

B, T, C = 2, 2048, 2048
NH, NKV, HD = 16, 4, 128
DFF = 4 * C
TQ = 512          # tokens per core
NT = C // 128     # 16 feature tiles
NF = DFF // 128   # 64 ff tiles
EPS = 1.1920929e-07
NCORES = 8

_CACHE = None


def _build():
    import concourse.bass as bass
    import concourse.tile as tile
    from concourse import mybir, bacc
    from concourse.masks import make_identity

    dt = mybir.dt
    f32, bf16, fp8 = dt.float32, dt.bfloat16, dt.float8e4
    Alu = mybir.AluOpType
    Act = mybir.ActivationFunctionType

    nc = bacc.Bacc("TRN2", target_bir_lowering=False, debug=False, num_devices=NCORES)

    for val in (EPS, HD * EPS):
        tns = nc.alloc_sbuf_tensor(f"const-f32-{val}", [128, 1], f32)
        nc.gpsimd.memset(tns.ap(), val)
        nc.const_aps.aps[(f32, val)] = tns.ap()
    nc.all_engine_barrier()

    xT = nc.declare_dram_parameter("xT", [C, TQ], f32, isOutput=False)
    csc = nc.declare_dram_parameter("csc", [128, TQ], f32, isOutput=False)
    css = nc.declare_dram_parameter("css", [128, TQ], f32, isOutput=False)
    mask = nc.declare_dram_parameter("mask", [T, TQ], fp8, isOutput=False)
    # all weights host-pretiled to [128, n_tiles*512] (16KB-contiguous rows)
    wq = nc.declare_dram_parameter("wq", [128, 4 * NT * TQ], bf16, isOutput=False)
    wk = nc.declare_dram_parameter("wk", [128, NT * TQ], bf16, isOutput=False)
    wv = nc.declare_dram_parameter("wv", [128, NT * TQ], bf16, isOutput=False)
    wo = nc.declare_dram_parameter("wo", [128, 4 * NT * TQ], bf16, isOutput=False)
    wfc = nc.declare_dram_parameter("wfc", [128, 16 * NT * TQ], bf16, isOutput=False)
    wproj = nc.declare_dram_parameter("wproj", [128, 4 * NF * TQ], bf16, isOutput=False)
    out_tm = nc.declare_dram_parameter("out", [TQ, C], f32, isOutput=True)

    ck_in = nc.dram_tensor("ck_in", [512, TQ], bf16)
    ck_out = nc.dram_tensor("ck_out", [2048, TQ], bf16)
    cv_in = nc.dram_tensor("cv_in", [512, TQ], bf16)
    cv_out = nc.dram_tensor("cv_out", [2048, TQ], bf16)

    with tile.TileContext(nc, num_cores=NCORES) as tc:
        with (
            tc.tile_pool(name="const", bufs=1) as constp,
            tc.tile_pool(name="persist", bufs=1) as pp,
            tc.tile_pool(name="work", bufs=3) as wpool,
            tc.tile_pool(name="wstream", bufs=3) as wsp,
        ):
            ident = constp.tile([128, 128], bf16, tag="ident")
            make_identity(nc, ident)
            ident_f = constp.tile([128, 128], f32, tag="identf")
            make_identity(nc, ident_f)
            ones = constp.tile([128, 1], bf16, tag="ones")
            nc.gpsimd.memset(ones, 1.0)
            csc_sb = constp.tile([128, TQ], f32, tag="csc")
            nc.sync.dma_start(csc_sb[:], csc[:])
            css_sb = constp.tile([128, TQ], f32, tag="css")
            nc.sync.dma_start(css_sb[:], css[:])

            # x_mid^T lives across attention + MLP
            xmT = pp.tile([128, NT, TQ], f32, tag="xmT")

            def norm_scale_row(ssq_ps, scale, bias, tag):
                """[1,TQ] psum sum-of-squares -> broadcast [128,TQ] f32 scale."""
                sr = wpool.tile([1, TQ], f32, tag="srow")
                nc.scalar.activation(sr[:], ssq_ps[:], Act.Sqrt, bias=bias, scale=scale)
                sb0 = wpool.tile([128, TQ], f32, tag=tag)
                nc.gpsimd.partition_broadcast(sb0[:], sr[:])
                sb = wpool.tile([128, TQ], f32, tag=tag)
                nc.vector.reciprocal(sb[:], sb0[:])
                return sb

            def load_xT(i, pool):
                xin = pool.tile([128, TQ], f32, tag="xin", bufs=2, name="xin")
                nc.sync.dma_start(xin[:], xT[128 * i:128 * (i + 1), :])
                return xin

            def rope(ps, pool):
                """psum [128,TQ] f32 -> rope'd bf16 sbuf tile."""
                raw = pool.tile([128, TQ], bf16, tag="rraw", bufs=3, name="rraw")
                nc.scalar.copy(raw[:], ps[:])
                sw = pool.tile([128, TQ], bf16, tag="rsw", bufs=2, name="rsw")
                nc.sync.dma_start(sw[0:64, :], raw[64:128, :])
                nc.sync.dma_start(sw[64:128, :], raw[0:64, :])
                rr = pool.tile([128, TQ], bf16, tag="rr", bufs=4, name="rr")
                nc.vector.tensor_tensor(rr[:], raw[:], csc_sb[:], Alu.mult)
                t2 = pool.tile([128, TQ], bf16, tag="rt2", bufs=2, name="rt2")
                nc.vector.tensor_tensor(t2[:], sw[:], css_sb[:], Alu.mult)
                nc.vector.tensor_tensor(rr[:], rr[:], t2[:], Alu.add)
                return rr

            def sumsq(rr, pool):
                sq = pool.tile([128, TQ], bf16, tag="rsq", bufs=4, name="rsq")
                nc.vector.tensor_tensor(sq[:], rr[:], rr[:], Alu.mult)
                return sq

            with tc.tile_pool(name="attn", bufs=1) as ap_:
                mask_sb = ap_.tile([128, NT, TQ], fp8, tag="mask_sb")
                nc.sync.dma_start(mask_sb[:], mask.rearrange("(g p) t -> p g t", p=128))
                qs_sb = ap_.tile([128, NH, TQ], bf16, tag="qs_sb")
                hT = ap_.tile([128, NT, TQ], bf16, tag="hT")
                vloc = ap_.tile([128, 4, TQ], bf16, tag="vloc")

                with tc.tile_pool(name="ps1", bufs=1, space="PSUM") as ps1:
                    # ---- P0: pre-attention rmsnorm (feature-major) ----
                    ssq_ps = ps1.tile([1, TQ], f32, tag="row", bufs=3)
                    for i in range(NT):
                        xin = load_xT(i, ap_)
                        xsq = wpool.tile([128, TQ], bf16, tag="xsq", bufs=6)
                        nc.vector.tensor_tensor(xsq[:], xin[:], xin[:], Alu.mult)
                        nc.tensor.matmul(ssq_ps[:], lhsT=ones[:], rhs=xsq[:],
                                         start=(i == 0), stop=(i == NT - 1))
                    s1b = norm_scale_row(ssq_ps, 1.0 / C, EPS, "sbcast")
                    for i in range(NT):
                        xin = load_xT(i, ap_)
                        nc.vector.tensor_tensor(hT[:, i], xin[:], s1b[:], Alu.mult)

                    # ---- K heads first: project + rope + k-norm -> cc_in ----
                    kps = [ps1.tile([128, TQ], f32, tag="qkv", bufs=4,
                                    name=f"kps_{_k}") for _k in range(4)]
                    wk_sb = wsp.tile([128, NT, TQ], bf16, tag="wslab", bufs=2,
                                     name="wk_sb")
                    nc.sync.dma_start(wk_sb[:], wk.rearrange("p (g t) -> p g t", t=TQ))
                    for k in range(4):
                        for i in range(NT):
                            nc.tensor.matmul(kps[k][:],
                                             lhsT=wk_sb[:, i, 128 * k:128 * (k + 1)],
                                             rhs=hT[:, i],
                                             start=(i == 0), stop=(i == NT - 1))
                    for kh in range(4):
                        rr = rope(kps[kh], ap_)
                        sq = sumsq(rr, ap_)
                        sps = ps1.tile([1, TQ], f32, tag="row", bufs=3)
                        nc.tensor.matmul(sps[:], lhsT=ones[:], rhs=sq[:],
                                         start=True, stop=True)
                        sb = norm_scale_row(sps, 1.0 / HD, EPS, "sbcast")
                        kt = ap_.tile([128, TQ], bf16, tag="ktile", bufs=3, name="kt")
                        nc.vector.tensor_tensor(kt[:], rr[:], sb[:], Alu.mult)
                        nc.sync.dma_start(ck_in[128 * kh:128 * (kh + 1), :], kt[:])

                    nc.gpsimd.collective_compute(
                        "AllGather", Alu.bypass,
                        replica_groups=[[0, 1, 2, 3], [4, 5, 6, 7]],
                        ins=[ck_in[:]], outs=[ck_out[:]])

                    # ---- V heads: project + transpose to token-major -> cv_in ----
                    vps = [ps1.tile([128, TQ], f32, tag="qkv", bufs=4,
                                    name=f"vps_{_k}") for _k in range(4)]
                    wv_sb = wsp.tile([128, NT, TQ], bf16, tag="wslab", bufs=2,
                                     name="wv_sb")
                    nc.sync.dma_start(wv_sb[:], wv.rearrange("p (g t) -> p g t", t=TQ))
                    for k in range(4):
                        for i in range(NT):
                            nc.tensor.matmul(vps[k][:],
                                             lhsT=wv_sb[:, i, 128 * k:128 * (k + 1)],
                                             rhs=hT[:, i],
                                             start=(i == 0), stop=(i == NT - 1))
                    for kh in range(4):
                        vb = ap_.tile([128, TQ], bf16, tag="ktile", bufs=3, name="vb")
                        nc.scalar.copy(vb[:], vps[kh][:])
                        for j in range(4):
                            tps = ps1.tile([128, 128], bf16, tag="tr", bufs=1)
                            nc.tensor.transpose(tps[:], vb[:, 128 * j:128 * (j + 1)],
                                                ident[:])
                            nc.vector.tensor_copy(
                                out=vloc[:, j, 128 * kh:128 * (kh + 1)], in_=tps[:])
                    for j in range(4):
                        nc.sync.dma_start(
                            cv_in[128 * j:128 * (j + 1), :], vloc[:, j, :])
                    nc.gpsimd.collective_compute(
                        "AllGather", Alu.bypass,
                        replica_groups=[[0, 1, 2, 3], [4, 5, 6, 7]],
                        ins=[cv_in[:]], outs=[cv_out[:]])

                    # ---- Q heads: project + rope + deferred q-norm ----
                    # ssq matmuls of group g are emitted after group g+1's
                    # projection matmuls so the PE never waits on the DVE chain
                    pending = None

                    def finish_q(pend):
                        hg, rrs, sqs = pend
                        for k in range(4):
                            h = 4 * hg + k
                            sps = ps1.tile([1, TQ], f32, tag="row", bufs=3)
                            nc.tensor.matmul(sps[:], lhsT=ones[:], rhs=sqs[k][:],
                                             start=True, stop=True)
                            sb = norm_scale_row(sps, 1.0, HD * EPS, "sbcast")
                            nc.vector.tensor_tensor(qs_sb[:, h], rrs[k][:], sb[:],
                                                    Alu.mult)

                    for hg in range(4):
                        qps = [ps1.tile([128, TQ], f32, tag="qkv", bufs=4,
                                        name=f"qps{hg}_{_k}") for _k in range(4)]
                        wq_sb = wsp.tile([128, NT, TQ], bf16, tag="wslab", bufs=2,
                                         name=f"wq_sb{hg}")
                        nc.sync.dma_start(
                            wq_sb[:],
                            wq[:, NT * TQ * hg:NT * TQ * (hg + 1)].rearrange(
                                "p (g t) -> p g t", t=TQ))
                        rrs = []
                        sqs = []
                        for k in range(4):
                            for i in range(NT):
                                nc.tensor.matmul(qps[k][:],
                                                 lhsT=wq_sb[:, i, 128 * k:128 * (k + 1)],
                                                 rhs=hT[:, i],
                                                 start=(i == 0), stop=(i == NT - 1))
                            rrs.append(rope(qps[k], ap_))
                            sqs.append(sumsq(rrs[k], ap_))
                        if pending is not None:
                            finish_q(pending)
                        pending = (hg, rrs, sqs)
                    finish_q(pending)

                # ---- load gathered K/V ----
                k_sb = ap_.tile([128, 16, TQ], bf16, tag="k_sb")   # (kh, g)
                v_sb = ap_.tile([128, 16, TQ], bf16, tag="v_sb")   # (g, j)
                for g in range(4):
                    for kh in range(4):
                        nc.sync.dma_start(
                            k_sb[:, 4 * kh + g],
                            ck_out[512 * g + 128 * kh:512 * g + 128 * (kh + 1), :])
                    for j in range(4):
                        nc.sync.dma_start(
                            v_sb[:, 4 * g + j],
                            cv_out[512 * g + 128 * j:512 * g + 128 * (j + 1), :])

                # ---- attention, 4 sibling q-heads per kv head together ----
                yT = ap_.tile([128, NH, TQ], bf16, tag="hT")
                with tc.tile_pool(name="ps2", bufs=1, space="PSUM") as ps2:
                    for kh in range(NKV):
                      for pr in range(2):
                        hs = [4 * kh + 2 * pr + k for k in range(2)]
                        den2 = ps2.tile([33, TQ], f32, tag="den", bufs=1)
                        y_ps = [ps2.tile([128, TQ], f32, tag="y", bufs=4,
                                         name=f"y{kh}{pr}_{_k}") for _k in range(2)]
                        fifo = []

                        def drain_one():
                            m0, k0, p0 = fifo.pop(0)
                            nc.tensor.matmul(den2[32 * k0:32 * k0 + 1, :],
                                             lhsT=ones[:], rhs=p0[:],
                                             start=(m0 == 0), stop=(m0 == 15))
                            nc.tensor.matmul(
                                y_ps[k0][:],
                                lhsT=v_sb[:, m0, 128 * kh:128 * (kh + 1)],
                                rhs=p0[:],
                                start=(m0 == 0), stop=(m0 == 15))

                        for m in range(16):
                            g, mm = divmod(m, 4)
                            for k in range(2):
                                sc_ps = ps2.tile([128, TQ], f32, tag="sc", bufs=3)
                                nc.tensor.matmul(
                                    sc_ps[:],
                                    lhsT=k_sb[:, 4 * kh + g, 128 * mm:128 * (mm + 1)],
                                    rhs=qs_sb[:, hs[k]], start=True, stop=True)
                                p_bf = ap_.tile([128, TQ], bf16, tag="p_bf",
                                                bufs=6, name="p_bf")
                                nc.scalar.activation(p_bf[:], sc_ps[:], Act.Exp)
                                nc.vector.tensor_tensor(p_bf[:], p_bf[:],
                                                        mask_sb[:, m], Alu.mult)
                                fifo.append((m, k, p_bf))
                                if len(fifo) > 4:
                                    drain_one()
                        while fifo:
                            drain_one()
                        drs = []
                        for k in range(2):
                            dr = wpool.tile([1, TQ], f32, tag="srow")
                            nc.scalar.copy(dr[:], den2[32 * k:32 * k + 1, :])
                            drs.append(dr)
                        for k in range(2):
                            yraw = wpool.tile([128, TQ], f32, tag="yraw", bufs=3)
                            nc.scalar.copy(yraw[:], y_ps[k][:])
                            dr = drs[k]
                            db0 = wpool.tile([128, TQ], f32, tag="sbcast")
                            nc.gpsimd.partition_broadcast(db0[:], dr[:])
                            db = wpool.tile([128, TQ], f32, tag="sbcast")
                            nc.vector.reciprocal(db[:], db0[:])
                            nc.vector.tensor_tensor(yT[:, hs[k]], yraw[:], db[:],
                                                    Alu.mult)

                    # ---- wo projection + residual (feature-major xmT) ----
                    for n4 in range(4):
                        att_ps = [ps2.tile([128, TQ], f32, tag="y", bufs=4,
                                           name=f"att{n4}_{_k}") for _k in range(4)]
                        wo_sb = wsp.tile([128, NT, TQ], bf16, tag="wslab", bufs=2,
                                         name=f"wo_sb{n4}")
                        nc.sync.dma_start(
                            wo_sb[:],
                            wo[:, NT * TQ * n4:NT * TQ * (n4 + 1)].rearrange(
                                "p (g t) -> p g t", t=TQ))
                        for k in range(4):
                            for h in range(NH):
                                nc.tensor.matmul(att_ps[k][:],
                                                 lhsT=wo_sb[:, h, 128 * k:128 * (k + 1)],
                                                 rhs=yT[:, h],
                                                 start=(h == 0), stop=(h == NH - 1))
                            n = 4 * n4 + k
                            xin = load_xT(n, ap_)
                            nc.vector.tensor_tensor(xmT[:, n], att_ps[k][:],
                                                    xin[:], Alu.add)
            # attn pool closed

            # ---- MLP ----
            with tc.tile_pool(name="mlp", bufs=1) as mp:
                h2T = mp.tile([128, NT, TQ], bf16, tag="h2T")
                a_sb = mp.tile([128, NF, TQ], bf16, tag="a_sb")
                xm_tm = mp.tile([128, 4, C], bf16, tag="xm_tm")  # token-major x_mid

                with tc.tile_pool(name="ps3", bufs=1, space="PSUM") as ps3:
                    ssq2 = ps3.tile([1, TQ], f32, tag="row", bufs=2)
                    _dummy = 0
                    for i in range(NT):
                        xsq = wpool.tile([128, TQ], bf16, tag="xsq", bufs=6)
                        nc.vector.tensor_tensor(xsq[:], xmT[:, i], xmT[:, i],
                                                Alu.mult)
                        nc.tensor.matmul(ssq2[:], lhsT=ones[:], rhs=xsq[:],
                                         start=(i == 0), stop=(i == NT - 1))
                    s2b = norm_scale_row(ssq2, 1.0 / C, EPS, "sbcast")
                    for i in range(NT):
                        nc.vector.tensor_tensor(h2T[:, i], xmT[:, i], s2b[:],
                                                Alu.mult)

                    # transpose xmT -> token-major for the final residual
                    for i in range(NT):
                        for j in range(4):
                            tp2 = ps3.tile([128, 128], f32, tag="mm", bufs=6)
                            nc.tensor.transpose(tp2[:],
                                                xmT[:, i, 128 * j:128 * (j + 1)],
                                                ident_f[:])
                            nc.vector.tensor_copy(
                                out=xm_tm[:, j, 128 * i:128 * (i + 1)], in_=tp2[:])

                # fc + relu^2 (feature-major a)
                with tc.tile_pool(name="ps3b", bufs=1, space="PSUM") as ps3b:
                    for jc in range(16):
                        f_ps = [ps3b.tile([128, TQ], f32, tag="mm", bufs=8,
                                          name=f"fps{jc}_{_k}") for _k in range(4)]
                        wfc_sb = wsp.tile([128, NT, TQ], bf16, tag="wslab", bufs=2,
                                          name=f"wfc_sb{jc}")
                        nc.sync.dma_start(
                            wfc_sb[:],
                            wfc[:, NT * TQ * jc:NT * TQ * (jc + 1)].rearrange(
                                "p (g t) -> p g t", t=TQ))
                        for jf in range(4):
                            for i in range(NT):
                                nc.tensor.matmul(
                                    f_ps[jf][:],
                                    lhsT=wfc_sb[:, i, 128 * jf:128 * (jf + 1)],
                                    rhs=h2T[:, i],
                                    start=(i == 0), stop=(i == NT - 1))
                            f = 4 * jc + jf
                            r_bf = wpool.tile([128, TQ], bf16, tag="r_bf")
                            nc.scalar.activation(r_bf[:], f_ps[jf][:], Act.Relu)
                            nc.vector.tensor_tensor(a_sb[:, f], r_bf[:], r_bf[:],
                                                    Alu.mult)

                # proj: lhsT = a tile (1 LDW : 2 MMs), token-major output
                with tc.tile_pool(name="ps4", bufs=1, space="PSUM") as ps4:
                    for n2 in range(2):
                        o_ps = [ps4.tile([128, TQ], f32, tag="o", bufs=8,
                                         name=f"ops{n2}_{_k}") for _k in range(8)]
                        for f8 in range(NF // 8):
                            wp_sb = wsp.tile([128, 16, TQ], bf16, tag="wslab",
                                             bufs=2, name=f"wp{n2}_{f8}")
                            base = NF * 2 * TQ * n2 + 8 * 2 * TQ * f8
                            nc.sync.dma_start(
                                wp_sb[:],
                                wproj[:, base:base + 16 * TQ].rearrange(
                                    "p (g t) -> p g t", t=TQ))
                            for fo in range(8):
                                f = 8 * f8 + fo
                                for tj in range(4):
                                    nc.tensor.matmul(
                                        o_ps[2 * tj][:],
                                        lhsT=a_sb[:, f, 128 * tj:128 * (tj + 1)],
                                        rhs=wp_sb[:, 2 * fo, :],
                                        start=(f == 0), stop=(f == NF - 1))
                                    nc.tensor.matmul(
                                        o_ps[2 * tj + 1][:],
                                        lhsT=a_sb[:, f, 128 * tj:128 * (tj + 1)],
                                        rhs=wp_sb[:, 2 * fo + 1, :],
                                        start=(f == 0), stop=(f == NF - 1))
                        for tj in range(4):
                            for half in range(2):
                                cstart = 1024 * n2 + 512 * half
                                ov = wpool.tile([128, TQ], f32, tag="yraw")
                                nc.vector.tensor_tensor(
                                    ov[:], o_ps[2 * tj + half][:],
                                    xm_tm[:, tj, cstart:cstart + 512], Alu.add)
                                nc.sync.dma_start(
                                    out_tm[128 * tj:128 * (tj + 1),
                                           cstart:cstart + 512], ov[:])

    nc.compile()
    return nc


def _make_in_maps(x, cos, sin, weights_b):
    import ml_dtypes
    cosT = cos[0, :, 0, :].T  # [64, T]
    sinT = sin[0, :, 0, :].T
    in_maps = []
    for c in range(NCORES):
        b, r = divmod(c, 4)
        sl = slice(TQ * r, TQ * (r + 1))
        qpos = np.arange(TQ * r, TQ * (r + 1))
        m = {
            "xT": np.ascontiguousarray(x[b, sl, :].T),
            "csc": np.ascontiguousarray(
                np.concatenate([cosT[:, sl], cosT[:, sl]], axis=0)),
            "css": np.ascontiguousarray(
                np.concatenate([sinT[:, sl], -sinT[:, sl]], axis=0)),
            "mask": (np.arange(T)[:, None] <= qpos[None, :]).astype(ml_dtypes.float8_e4m3),
        }
        m.update(weights_b)
        in_maps.append(m)
    return in_maps


def kernel(x, cos, sin, wq, wk, wv, wo, w_fc, w_proj):
    global _CACHE
    import ml_dtypes
    from concourse.bass_utils import run_bass_kernel_spmd

    bf = ml_dtypes.bfloat16
    x = np.asarray(x, np.float32)
    cos = np.asarray(cos, np.float32)
    sin = np.asarray(sin, np.float32)
    def tile_w(w, chunk):
        # [R, F] -> [128, (F//chunk) * (R//128) * chunk]: per output column
        # chunk, row-blocks become contiguous along the free axis
        R, F = w.shape
        t = w.reshape(R // 128, 128, F // chunk, chunk)
        t = t.transpose(1, 2, 0, 3)  # [128, F//chunk, R//128, chunk]
        return np.ascontiguousarray(t.reshape(128, -1)).astype(bf)

    wproj_f = np.asarray(w_proj, np.float32)
    # proj layout: [n2][f-tile][2 halves of 512]: order free axis as
    # (n2, f, half, 512): build [128, 2*64*2*512]
    tpj = wproj_f.reshape(NF, 128, 2, 2, TQ)       # [f, p, n2, half, t]
    tpj = tpj.transpose(1, 2, 0, 3, 4)             # [p, n2, f, half, t]
    wproj_t = np.ascontiguousarray(tpj.reshape(128, -1)).astype(bf)

    weights_b = {
        "wq": tile_w(np.asarray(wq, np.float32), TQ),
        "wk": tile_w(np.asarray(wk, np.float32), NKV * HD),
        "wv": tile_w(np.asarray(wv, np.float32), NKV * HD),
        "wo": tile_w(np.asarray(wo, np.float32), TQ),
        "wfc": tile_w(np.asarray(w_fc, np.float32), TQ),
        "wproj": wproj_t,
    }

    if _CACHE is None:
        _CACHE = _build()
    nc = _CACHE

    in_maps = _make_in_maps(x, cos, sin, weights_b)
    res = run_bass_kernel_spmd(nc, in_maps, list(range(NCORES)))
    out = np.empty((B, T, C), np.float32)
    for c in range(NCORES):
        b, r = divmod(c, 4)
        out[b, TQ * r:TQ * (r + 1), :] = res.results[c]["out"]
    return out



# revision 3
# speedup vs baseline: 1.0324x; 1.0324x over previous
"""Trainium2 Bass kernel for nn_Block_32762010534337 (dense transformer block).

Strategy: sequence-parallel over 8 cores. Core c owns 512 tokens (batch c//4,
token chunk c%4). Each core projects K/V from raw x (the pre-norm rmsnorm scale
commutes through the linear projections: rotary is per-token linear and the
q/k rmsnorms are scale-invariant, so only V needs the explicit scale),
AllGathers K/V within its batch group of 4 cores (overlapped with the Q
projections), then runs causal attention + wo + MLP (relu^2) for its 512
tokens with fully replicated bf16 weights. All rsqrt/reciprocal are computed
on the scalar engine as Exp(-a*Ln(x)) row ops to keep the DVE free.
Activations stay feature-major throughout; the host transposes per-core
inputs and the final output.
"""
import sys
import os

if "/opt/trn_rl_repo" not in sys.path:
    sys.path.insert(0, "/opt/trn_rl_repo")

import numpy as np

B, T, C = 2, 2048, 2048
NH, NKV, HD = 16, 4, 128
DFF = 4 * C
TQ = 512          # tokens per core
NT = C // 128     # 16 feature tiles
NF = DFF // 128   # 64 ff tiles
EPS = 1.1920929e-07
NCORES = 8

_CACHE = None


def _build():
    import concourse.bass as bass
    import concourse.tile as tile
    from concourse import mybir, bacc
    from concourse.masks import make_identity

    dt = mybir.dt
    f32, bf16 = dt.float32, dt.bfloat16
    Alu = mybir.AluOpType
    Act = mybir.ActivationFunctionType

    nc = bacc.Bacc("TRN2", target_bir_lowering=False, debug=False, num_devices=NCORES)

    for val in (0.0, EPS, HD * EPS):
        tns = nc.alloc_sbuf_tensor(f"const-f32-{val}", [128, 1], f32)
        nc.gpsimd.memset(tns.ap(), val)
        nc.const_aps.aps[(f32, val)] = tns.ap()
    nc.all_engine_barrier()

    xT = nc.declare_dram_parameter("xT", [C, TQ], bf16, isOutput=False)
    csc = nc.declare_dram_parameter("csc", [128, TQ], bf16, isOutput=False)
    css = nc.declare_dram_parameter("css", [128, TQ], bf16, isOutput=False)
    mask = nc.declare_dram_parameter("mask", [T, TQ], bf16, isOutput=False)
    # all weights host-pretiled to [128, n_tiles*512] (16KB-contiguous rows)
    wq = nc.declare_dram_parameter("wq", [128, 4 * NT * TQ], bf16, isOutput=False)
    wk = nc.declare_dram_parameter("wk", [128, NT * TQ], bf16, isOutput=False)
    wv = nc.declare_dram_parameter("wv", [128, NT * TQ], bf16, isOutput=False)
    wo = nc.declare_dram_parameter("wo", [128, 4 * NT * TQ], bf16, isOutput=False)
    wfc = nc.declare_dram_parameter("wfc", [128, 16 * NT * TQ], bf16, isOutput=False)
    wproj = nc.declare_dram_parameter("wproj", [128, 16 * NF * 128], bf16,
                                      isOutput=False)
    out_fm = nc.declare_dram_parameter("out", [C, TQ], f32, isOutput=True)

    ck_in = nc.dram_tensor("ck_in", [512, TQ], bf16)
    ck_out = nc.dram_tensor("ck_out", [2048, TQ], bf16)
    cv_in = nc.dram_tensor("cv_in", [512, TQ], bf16)
    cv_out = nc.dram_tensor("cv_out", [2048, TQ], bf16)

    with tile.TileContext(nc, num_cores=NCORES) as tc:
        with (
            tc.tile_pool(name="const", bufs=1) as constp,
            tc.tile_pool(name="persist", bufs=1) as pp,
            tc.tile_pool(name="work", bufs=3) as wpool,
            tc.tile_pool(name="wstream", bufs=3) as wsp,
        ):
            ident = constp.tile([128, 128], bf16, tag="ident")
            make_identity(nc, ident)
            ones = constp.tile([128, 1], bf16, tag="ones")
            nc.gpsimd.memset(ones, 1.0)
            csc_sb = constp.tile([128, TQ], bf16, tag="csc")
            nc.sync.dma_start(csc_sb[:], csc[:])
            css_sb = constp.tile([128, TQ], bf16, tag="css")
            nc.sync.dma_start(css_sb[:], css[:])

            # x (bf16) resident across the whole kernel; x_mid^T for MLP
            xin = pp.tile([128, NT, TQ], bf16, tag="xin")
            nc.sync.dma_start(xin[:], xT.rearrange("(g p) t -> p g t", p=128))
            xmT = pp.tile([128, NT, TQ], bf16, tag="xmT")

            def rsqrt_row(ssq_ps, scale, bias, neg_half=-0.5):
                """[1,TQ] psum sum-of-squares -> broadcast [128,TQ] f32
                1/sqrt(scale*x+bias), via Exp(-0.5*Ln(x)) on the scalar eng."""
                ln = wpool.tile([1, TQ], f32, tag="srow", bufs=4, name="srow")
                nc.scalar.activation(ln[:], ssq_ps[:], Act.Ln, bias=bias,
                                     scale=scale)
                r = wpool.tile([1, TQ], f32, tag="srow", bufs=4, name="srow2")
                nc.scalar.activation(r[:], ln[:], Act.Exp, scale=neg_half)
                sb = wpool.tile([128, TQ], f32, tag="sbcast")
                nc.gpsimd.partition_broadcast(sb[:], r[:])
                return sb

            def rope(ps, pool):
                """psum [128,TQ] f32 -> rope'd bf16 sbuf tile."""
                raw = pool.tile([128, TQ], bf16, tag="rraw", bufs=3, name="rraw")
                nc.scalar.copy(raw[:], ps[:])
                sw = pool.tile([128, TQ], bf16, tag="rsw", bufs=2, name="rsw")
                nc.sync.dma_start(sw[0:64, :], raw[64:128, :])
                nc.sync.dma_start(sw[64:128, :], raw[0:64, :])
                rr = pool.tile([128, TQ], bf16, tag="rr", bufs=4, name="rr")
                nc.vector.tensor_tensor(rr[:], raw[:], csc_sb[:], Alu.mult)
                t2 = pool.tile([128, TQ], bf16, tag="rt2", bufs=2, name="rt2")
                nc.vector.tensor_tensor(t2[:], sw[:], css_sb[:], Alu.mult)
                nc.vector.tensor_tensor(rr[:], rr[:], t2[:], Alu.add)
                return rr

            def sumsq(rr, pool):
                sq = pool.tile([128, TQ], bf16, tag="rsq", bufs=4, name="rsq")
                nc.vector.tensor_tensor(sq[:], rr[:], rr[:], Alu.mult)
                return sq

            with tc.tile_pool(name="attn", bufs=1) as ap_:
                mask_sb = ap_.tile([128, NT, TQ], bf16, tag="mask_sb")
                nc.sync.dma_start(mask_sb[:], mask.rearrange("(g p) t -> p g t", p=128))
                qs_sb = ap_.tile([128, NH, TQ], bf16, tag="qs_sb")
                vloc = ap_.tile([128, 4, TQ], bf16, tag="vloc")

                with tc.tile_pool(name="ps1", bufs=1, space="PSUM") as ps1:
                    # ---- K heads first: project raw x + rope + k-norm ----
                    kps = [ps1.tile([128, TQ], f32, tag="qkv", bufs=4,
                                    name=f"kps_{_k}") for _k in range(4)]
                    wk_sb = wsp.tile([128, NT, TQ], bf16, tag="wslab", bufs=2,
                                     name="wk_sb")
                    nc.sync.dma_start(wk_sb[:], wk.rearrange("p (g t) -> p g t", t=TQ))
                    for k in range(4):
                        for i in range(NT):
                            nc.tensor.matmul(kps[k][:],
                                             lhsT=wk_sb[:, i, 128 * k:128 * (k + 1)],
                                             rhs=xin[:, i],
                                             start=(i == 0), stop=(i == NT - 1))
                    for kh in range(4):
                        rr = rope(kps[kh], ap_)
                        sq = sumsq(rr, ap_)
                        sps = ps1.tile([1, TQ], f32, tag="row", bufs=3)
                        nc.tensor.matmul(sps[:], lhsT=ones[:], rhs=sq[:],
                                         start=True, stop=True)
                        sb = rsqrt_row(sps, 1.0 / HD, EPS)
                        kt = ap_.tile([128, TQ], bf16, tag="ktile", bufs=3, name="kt")
                        nc.vector.tensor_tensor(kt[:], rr[:], sb[:], Alu.mult)
                        nc.sync.dma_start(ck_in[128 * kh:128 * (kh + 1), :], kt[:])

                    nc.gpsimd.collective_compute(
                        "AllGather", Alu.bypass,
                        replica_groups=[[0, 1, 2, 3], [4, 5, 6, 7]],
                        ins=[ck_in[:]], outs=[ck_out[:]])

                    # ---- pre-norm sum-of-squares (only V needs the scale) ----
                    ssq_ps = ps1.tile([1, TQ], f32, tag="row", bufs=3)
                    for i in range(NT):
                        xsq = wpool.tile([128, TQ], bf16, tag="xsq", bufs=6)
                        nc.vector.tensor_tensor(xsq[:], xin[:, i], xin[:, i],
                                                Alu.mult)
                        nc.tensor.matmul(ssq_ps[:], lhsT=ones[:], rhs=xsq[:],
                                         start=(i == 0), stop=(i == NT - 1))
                    s1b = rsqrt_row(ssq_ps, 1.0 / C, EPS)

                    # ---- V heads: project + scale + transpose to token-major ----
                    vps = [ps1.tile([128, TQ], f32, tag="qkv", bufs=4,
                                    name=f"vps_{_k}") for _k in range(4)]
                    wv_sb = wsp.tile([128, NT, TQ], bf16, tag="wslab", bufs=2,
                                     name="wv_sb")
                    nc.sync.dma_start(wv_sb[:], wv.rearrange("p (g t) -> p g t", t=TQ))
                    for k in range(4):
                        for i in range(NT):
                            nc.tensor.matmul(vps[k][:],
                                             lhsT=wv_sb[:, i, 128 * k:128 * (k + 1)],
                                             rhs=xin[:, i],
                                             start=(i == 0), stop=(i == NT - 1))
                    for kh in range(4):
                        vb = ap_.tile([128, TQ], bf16, tag="ktile", bufs=3, name="vb")
                        nc.vector.tensor_tensor(vb[:], vps[kh][:], s1b[:],
                                                Alu.mult)
                        for j in range(4):
                            tps = ps1.tile([128, 128], bf16, tag="tr", bufs=1)
                            nc.tensor.transpose(tps[:], vb[:, 128 * j:128 * (j + 1)],
                                                ident[:])
                            nc.vector.tensor_copy(
                                out=vloc[:, j, 128 * kh:128 * (kh + 1)], in_=tps[:])
                    for j in range(4):
                        nc.sync.dma_start(
                            cv_in[128 * j:128 * (j + 1), :], vloc[:, j, :])
                    nc.gpsimd.collective_compute(
                        "AllGather", Alu.bypass,
                        replica_groups=[[0, 1, 2, 3], [4, 5, 6, 7]],
                        ins=[cv_in[:]], outs=[cv_out[:]])

                    # ---- Q heads: project + rope + deferred q-norm ----
                    # ssq matmuls of group g are emitted after group g+1's
                    # projection matmuls so the PE never waits on the DVE chain
                    pending = None

                    def finish_q(pend):
                        hg, rrs, sqs = pend
                        for k in range(4):
                            h = 4 * hg + k
                            sps = ps1.tile([1, TQ], f32, tag="row", bufs=3)
                            nc.tensor.matmul(sps[:], lhsT=ones[:], rhs=sqs[k][:],
                                             start=True, stop=True)
                            sb = rsqrt_row(sps, 1.0, HD * EPS)
                            nc.vector.tensor_tensor(qs_sb[:, h], rrs[k][:], sb[:],
                                                    Alu.mult)

                    for hg in range(4):
                        qps = [ps1.tile([128, TQ], f32, tag="qkv", bufs=4,
                                        name=f"qps{hg}_{_k}") for _k in range(4)]
                        wq_sb = wsp.tile([128, NT, TQ], bf16, tag="wslab", bufs=2,
                                         name=f"wq_sb{hg}")
                        nc.sync.dma_start(
                            wq_sb[:],
                            wq[:, NT * TQ * hg:NT * TQ * (hg + 1)].rearrange(
                                "p (g t) -> p g t", t=TQ))
                        rrs = []
                        sqs = []
                        for k in range(4):
                            for i in range(NT):
                                nc.tensor.matmul(qps[k][:],
                                                 lhsT=wq_sb[:, i, 128 * k:128 * (k + 1)],
                                                 rhs=xin[:, i],
                                                 start=(i == 0), stop=(i == NT - 1))
                            rrs.append(rope(qps[k], ap_))
                            sqs.append(sumsq(rrs[k], ap_))
                        if pending is not None:
                            finish_q(pending)
                        pending = (hg, rrs, sqs)
                    finish_q(pending)

                # ---- load gathered K/V ----
                k_sb = ap_.tile([128, 16, TQ], bf16, tag="k_sb")   # (kh, g)
                v_sb = ap_.tile([128, 16, TQ], bf16, tag="v_sb")   # (g, j)
                for g in range(4):
                    for kh in range(4):
                        nc.sync.dma_start(
                            k_sb[:, 4 * kh + g],
                            ck_out[512 * g + 128 * kh:512 * g + 128 * (kh + 1), :])
                    for j in range(4):
                        nc.sync.dma_start(
                            v_sb[:, 4 * g + j],
                            cv_out[512 * g + 128 * j:512 * g + 128 * (j + 1), :])

                # ---- attention, 2 sibling q-heads per (kv head, pair) ----
                yT = ap_.tile([128, NH, TQ], bf16, tag="yT")
                with tc.tile_pool(name="ps2", bufs=1, space="PSUM") as ps2:
                    pending_epi = None

                    def emit_epi(epi):
                        hs, den2, y_ps = epi
                        for k in range(2):
                            ln = wpool.tile([1, TQ], f32, tag="srow", bufs=4,
                                            name="eln")
                            nc.scalar.activation(ln[:], den2[32 * k:32 * k + 1, :],
                                                 Act.Ln)
                            r = wpool.tile([1, TQ], f32, tag="srow", bufs=4,
                                           name="erow")
                            nc.scalar.activation(r[:], ln[:], Act.Exp, scale=-1.0)
                            db = wpool.tile([128, TQ], f32, tag="sbcast")
                            nc.gpsimd.partition_broadcast(db[:], r[:])
                            nc.vector.tensor_tensor(yT[:, hs[k]], y_ps[k][:],
                                                    db[:], Alu.mult)

                    for kh in range(NKV):
                      for pr in range(2):
                        hs = [4 * kh + 2 * pr + k for k in range(2)]
                        den2 = ps2.tile([33, TQ], f32, tag="den", bufs=2)
                        y_ps = [ps2.tile([128, TQ], f32, tag="y", bufs=3,
                                         name=f"y{kh}{pr}_{_k}") for _k in range(2)]
                        fifo = []

                        def drain_one():
                            m0, k0, p0 = fifo.pop(0)
                            nc.tensor.matmul(den2[32 * k0:32 * k0 + 1, :],
                                             lhsT=ones[:], rhs=p0[:],
                                             start=(m0 == 0), stop=(m0 == 15))
                            nc.tensor.matmul(
                                y_ps[k0][:],
                                lhsT=v_sb[:, m0, 128 * kh:128 * (kh + 1)],
                                rhs=p0[:],
                                start=(m0 == 0), stop=(m0 == 15))

                        for m in range(16):
                            g, mm = divmod(m, 4)
                            for k in range(2):
                                sc_ps = ps2.tile([128, TQ], f32, tag="sc", bufs=3)
                                nc.tensor.matmul(
                                    sc_ps[:],
                                    lhsT=k_sb[:, 4 * kh + g, 128 * mm:128 * (mm + 1)],
                                    rhs=qs_sb[:, hs[k]], start=True, stop=True)
                                p_bf = ap_.tile([128, TQ], bf16, tag="p_bf",
                                                bufs=6, name="p_bf")
                                nc.scalar.activation(p_bf[:], sc_ps[:], Act.Exp)
                                nc.vector.tensor_tensor(p_bf[:], p_bf[:],
                                                        mask_sb[:, m], Alu.mult)
                                fifo.append((m, k, p_bf))
                                if len(fifo) > 4:
                                    drain_one()
                            if m == 1 and pending_epi is not None:
                                emit_epi(pending_epi)
                                pending_epi = None
                        while fifo:
                            drain_one()
                        pending_epi = (hs, den2, y_ps)
                    emit_epi(pending_epi)

                # ---- wo projection + residual (feature-major xmT) ----
                with tc.tile_pool(name="ps2b", bufs=1, space="PSUM") as ps2b:
                    for n4 in range(4):
                        att_ps = [ps2b.tile([128, TQ], f32, tag="att", bufs=8,
                                            name=f"att{n4}_{_k}") for _k in range(4)]
                        wo_sb = wsp.tile([128, NT, TQ], bf16, tag="wslab", bufs=2,
                                         name=f"wo_sb{n4}")
                        nc.sync.dma_start(
                            wo_sb[:],
                            wo[:, NT * TQ * n4:NT * TQ * (n4 + 1)].rearrange(
                                "p (g t) -> p g t", t=TQ))
                        for k in range(4):
                            for h in range(NH):
                                nc.tensor.matmul(att_ps[k][:],
                                                 lhsT=wo_sb[:, h, 128 * k:128 * (k + 1)],
                                                 rhs=yT[:, h],
                                                 start=(h == 0), stop=(h == NH - 1))
                            n = 4 * n4 + k
                            nc.vector.tensor_tensor(xmT[:, n], att_ps[k][:],
                                                    xin[:, n], Alu.add)
            # attn pool closed

            # ---- MLP ----
            with tc.tile_pool(name="mlp", bufs=1) as mp:
                h2T = mp.tile([128, NT, TQ], bf16, tag="h2T")
                a_sb = mp.tile([128, NF, TQ], bf16, tag="a_sb")

                with tc.tile_pool(name="ps3", bufs=1, space="PSUM") as ps3:
                    ssq2 = ps3.tile([1, TQ], f32, tag="row", bufs=2)
                    for i in range(NT):
                        xsq = wpool.tile([128, TQ], bf16, tag="xsq", bufs=6)
                        nc.vector.tensor_tensor(xsq[:], xmT[:, i], xmT[:, i],
                                                Alu.mult)
                        nc.tensor.matmul(ssq2[:], lhsT=ones[:], rhs=xsq[:],
                                         start=(i == 0), stop=(i == NT - 1))
                    s2b = rsqrt_row(ssq2, 1.0 / C, EPS)
                    for i in range(NT):
                        nc.vector.tensor_tensor(h2T[:, i], xmT[:, i], s2b[:],
                                                Alu.mult)

                # fc + relu^2 (feature-major a)
                with tc.tile_pool(name="ps3b", bufs=1, space="PSUM") as ps3b:
                    for jc in range(16):
                        f_ps = [ps3b.tile([128, TQ], f32, tag="mm", bufs=8,
                                          name=f"fps{jc}_{_k}") for _k in range(4)]
                        wfc_sb = wsp.tile([128, NT, TQ], bf16, tag="wslab", bufs=2,
                                          name=f"wfc_sb{jc}")
                        nc.sync.dma_start(
                            wfc_sb[:],
                            wfc[:, NT * TQ * jc:NT * TQ * (jc + 1)].rearrange(
                                "p (g t) -> p g t", t=TQ))
                        for jf in range(4):
                            for i in range(NT):
                                nc.tensor.matmul(
                                    f_ps[jf][:],
                                    lhsT=wfc_sb[:, i, 128 * jf:128 * (jf + 1)],
                                    rhs=h2T[:, i],
                                    start=(i == 0), stop=(i == NT - 1))
                            f = 4 * jc + jf
                            r_bf = wpool.tile([128, TQ], bf16, tag="r_bf")
                            nc.scalar.activation(r_bf[:], f_ps[jf][:], Act.Relu)
                            nc.vector.tensor_tensor(a_sb[:, f], r_bf[:], r_bf[:],
                                                    Alu.mult)

                # proj: weight-stationary, feature-major output + residual
                with tc.tile_pool(name="ps4", bufs=1, space="PSUM") as ps4:
                    for cg in range(4):
                        o_ps = [ps4.tile([128, TQ], f32, tag="o", bufs=8,
                                         name=f"ops{cg}_{_k}") for _k in range(4)]
                        for f8 in range(4):
                            wp_sb = wsp.tile([128, 16, TQ], bf16, tag="wslab",
                                             bufs=2, name=f"wp{cg}_{f8}")
                            base = (cg * 4 + f8) * 16 * TQ
                            nc.sync.dma_start(
                                wp_sb[:],
                                wproj[:, base:base + 16 * TQ].rearrange(
                                    "p (g t) -> p g t", t=TQ))
                            for fl in range(16):
                                f = 16 * f8 + fl
                                for cl in range(4):
                                    nc.tensor.matmul(
                                        o_ps[cl][:],
                                        lhsT=wp_sb[:, fl, 128 * cl:128 * (cl + 1)],
                                        rhs=a_sb[:, f],
                                        start=(f == 0), stop=(f == NF - 1))
                        for cl in range(4):
                            c = 4 * cg + cl
                            ov = wpool.tile([128, TQ], f32, tag="yraw")
                            nc.vector.tensor_tensor(ov[:], o_ps[cl][:],
                                                    xmT[:, c], Alu.add)
                            nc.sync.dma_start(
                                out_fm[128 * c:128 * (c + 1), :], ov[:])

    nc.compile()
    return nc


def _make_in_maps(x, cos, sin, weights_b):
    import ml_dtypes
    bf = ml_dtypes.bfloat16
    cosT = cos[0, :, 0, :].T  # [64, T]
    sinT = sin[0, :, 0, :].T
    cscf = np.concatenate([cosT, cosT], axis=0)   # [128, T]
    cssf = np.concatenate([sinT, -sinT], axis=0)
    in_maps = []
    for c in range(NCORES):
        b, r = divmod(c, 4)
        sl = slice(TQ * r, TQ * (r + 1))
        qpos = np.arange(TQ * r, TQ * (r + 1))
        m = {
            "xT": np.ascontiguousarray(x[b, sl, :].T).astype(bf),
            "csc": np.ascontiguousarray(cscf[:, sl]).astype(bf),
            "css": np.ascontiguousarray(cssf[:, sl]).astype(bf),
            "mask": (np.arange(T)[:, None] <= qpos[None, :]).astype(bf),
        }
        m.update(weights_b)
        in_maps.append(m)
    return in_maps


def _prep_weights(wq, wk, wv, wo, w_fc, w_proj):
    import ml_dtypes
    bf = ml_dtypes.bfloat16

    def tile_w(w, chunk):
        # [R, F] -> [128, (F//chunk) * (R//128) * chunk]: per output column
        # chunk, row-blocks become contiguous along the free axis
        R, F = w.shape
        t = w.reshape(R // 128, 128, F // chunk, chunk)
        t = t.transpose(1, 2, 0, 3)  # [128, F//chunk, R//128, chunk]
        return np.ascontiguousarray(t.reshape(128, -1)).astype(bf)

    wproj_f = np.asarray(w_proj, np.float32)
    # proj layout: slabs of (cg, f8): [p, cg, f8, f_local 16, c_local 4, 128]
    t = wproj_f.reshape(4, 16, 128, 4, 4, 128)  # [f8, fl, p, cg, cl, col]
    t = t.transpose(2, 3, 0, 1, 4, 5)           # [p, cg, f8, fl, cl, col]
    wproj_t = np.ascontiguousarray(t.reshape(128, -1)).astype(bf)

    return {
        "wq": tile_w(np.asarray(wq, np.float32), TQ),
        "wk": tile_w(np.asarray(wk, np.float32), NKV * HD),
        "wv": tile_w(np.asarray(wv, np.float32), NKV * HD),
        "wo": tile_w(np.asarray(wo, np.float32), TQ),
        "wfc": tile_w(np.asarray(w_fc, np.float32), TQ),
        "wproj": wproj_t,
    }


def kernel(x, cos, sin, wq, wk, wv, wo, w_fc, w_proj):
    global _CACHE
    from concourse.bass_utils import run_bass_kernel_spmd

    x = np.asarray(x, np.float32)
    cos = np.asarray(cos, np.float32)
    sin = np.asarray(sin, np.float32)
    weights_b = _prep_weights(wq, wk, wv, wo, w_fc, w_proj)

    if _CACHE is None:
        _CACHE = _build()
    nc = _CACHE

    in_maps = _make_in_maps(x, cos, sin, weights_b)
    res = run_bass_kernel_spmd(nc, in_maps, list(range(NCORES)))
    out = np.empty((B, T, C), np.float32)
    for c in range(NCORES):
        b, r = divmod(c, 4)
        out[b, TQ * r:TQ * (r + 1), :] = res.results[c]["out"].T
    return out


# revision 15
# speedup vs baseline: 1.1158x; 1.0808x over previous
"""Trainium2 Bass kernel for nn_Block_32762010534337 (dense transformer block).

Strategy: sequence-parallel over 8 cores. Core c owns 512 tokens (batch c//4,
token chunk c%4). Each core projects K/V from raw x (the pre-norm rmsnorm scale
commutes through the linear projections: rotary is per-token linear and the
q/k rmsnorms are scale-invariant, so only V needs the explicit scale),
AllGathers K/V within its batch group of 4 cores (overlapped with the Q
projections), then runs causal attention + wo + MLP (relu^2) for its 512
tokens with fully replicated bf16 weights. All rsqrt/reciprocal are computed
on the scalar engine as Exp(-a*Ln(x)) row ops to keep the DVE free.
Activations stay feature-major throughout; the host transposes per-core
inputs and the final output.
"""
import sys
import os

if "/opt/trn_rl_repo" not in sys.path:
    sys.path.insert(0, "/opt/trn_rl_repo")

import numpy as np

B, T, C = 2, 2048, 2048
NH, NKV, HD = 16, 4, 128
DFF = 4 * C
TQ = 512          # tokens per core
NT = C // 128     # 16 feature tiles
NF = DFF // 128   # 64 ff tiles
EPS = 1.1920929e-07
NCORES = 8

_CACHE = None


def _build():
    import concourse.bass as bass
    import concourse.tile as tile
    from concourse import mybir, bacc
    from concourse.masks import make_identity

    dt = mybir.dt
    f32, bf16 = dt.float32, dt.bfloat16
    Alu = mybir.AluOpType
    Act = mybir.ActivationFunctionType

    nc = bacc.Bacc("TRN2", target_bir_lowering=False, debug=False, num_devices=NCORES)

    for val in (0.0, EPS, HD * EPS):
        tns = nc.alloc_sbuf_tensor(f"const-f32-{val}", [128, 1], f32)
        nc.gpsimd.memset(tns.ap(), val)
        nc.const_aps.aps[(f32, val)] = tns.ap()
    nc.all_engine_barrier()

    xT = nc.declare_dram_parameter("xT", [C, TQ], bf16, isOutput=False)
    csc = nc.declare_dram_parameter("csc", [128, TQ], bf16, isOutput=False)
    css = nc.declare_dram_parameter("css", [128, TQ], bf16, isOutput=False)
    mask = nc.declare_dram_parameter("mask", [T, TQ], bf16, isOutput=False)
    # all weights host-pretiled to [128, n_tiles*512] (16KB-contiguous rows)
    wq = nc.declare_dram_parameter("wq", [128, 4 * NT * TQ], bf16, isOutput=False)
    wk = nc.declare_dram_parameter("wk", [128, NT * TQ], bf16, isOutput=False)
    wv = nc.declare_dram_parameter("wv", [128, NT * TQ], bf16, isOutput=False)
    wo = nc.declare_dram_parameter("wo", [128, 4 * NT * TQ], bf16, isOutput=False)
    wfc = nc.declare_dram_parameter("wfc", [128, 16 * NT * TQ], bf16, isOutput=False)
    wproj = nc.declare_dram_parameter("wproj", [128, 16 * NF * 128], bf16,
                                      isOutput=False)
    out_fm = nc.declare_dram_parameter("out", [C, TQ], f32, isOutput=True)

    ck_in = nc.dram_tensor("ck_in", [512, TQ], bf16)
    ck_out = nc.dram_tensor("ck_out", [2048, TQ], bf16)
    cv_in = nc.dram_tensor("cv_in", [512, TQ], bf16)
    cv_out = nc.dram_tensor("cv_out", [2048, TQ], bf16)

    with tile.TileContext(nc, num_cores=NCORES) as tc:
        with (
            tc.tile_pool(name="const", bufs=1) as constp,
            tc.tile_pool(name="persist", bufs=1) as pp,
            tc.tile_pool(name="work", bufs=3) as wpool,
            tc.tile_pool(name="wstream", bufs=3) as wsp,
        ):
            ident = constp.tile([128, 128], bf16, tag="ident")
            make_identity(nc, ident)
            ones = constp.tile([128, 1], bf16, tag="ones")
            nc.gpsimd.memset(ones, 1.0)
            # one-hot column selectors: e4[:, j, :] is [128,97] with col 32j
            # ones; e2[:, k, :] is [128,33] with col 32k ones. Used to land
            # ssq rows of different heads on 32-aligned PSUM partitions
            # (single-partition access must be 32-aligned) so one batched DVE
            # reciprocal covers a whole group of rows.
            e4 = constp.tile([128, 4, 97], bf16, tag="e4")
            nc.gpsimd.memset(e4, 0.0)
            for j in range(4):
                nc.gpsimd.memset(e4[:, j, 32 * j:32 * j + 1], 1.0)
            e2 = constp.tile([128, 2, 33], bf16, tag="e2")
            nc.gpsimd.memset(e2, 0.0)
            for k in range(2):
                nc.gpsimd.memset(e2[:, k, 32 * k:32 * k + 1], 1.0)
            csc_sb = constp.tile([128, TQ], bf16, tag="csc")
            nc.sync.dma_start(csc_sb[:], csc[:])
            css_sb = constp.tile([128, TQ], bf16, tag="css")
            nc.sync.dma_start(css_sb[:], css[:])

            # x (bf16) resident across the whole kernel; x_mid^T for MLP
            xin = pp.tile([128, NT, TQ], bf16, tag="xin")
            for i in range(NT):
                nc.sync.dma_start(xin[:, i], xT[128 * i:128 * (i + 1), :])
            xmT = pp.tile([128, NT, TQ], bf16, tag="xmT")

            def rsqrt_rows(ssq_ps, scale, bias, nrows):
                """[nrows,TQ] psum sum-of-squares -> [nrows,TQ] f32 sbuf
                1/sqrt(scale*x+bias): one Sqrt + one batched DVE recip."""
                sr = wpool.tile([nrows, TQ], f32, tag="srow", bufs=3,
                                name="srow")
                nc.scalar.activation(sr[:], ssq_ps[:], Act.Sqrt, bias=bias,
                                     scale=scale)
                rr_ = wpool.tile([nrows, TQ], f32, tag="srow", bufs=3,
                                 name="srow2")
                nc.vector.reciprocal(rr_[:], sr[:])
                return rr_

            def bcast_row(rows, j, w=TQ):
                # partition_broadcast reads partition 0 only: stage non-zero
                # rows down to a partition-0 tile via SBUF->SBUF DMA first.
                if j == 0:
                    row = rows[0:1, :]
                else:
                    stage = wpool.tile([1, w], f32, tag="stage", bufs=2,
                                       name="stage")
                    nc.sync.dma_start(stage[:], rows[32 * j:32 * j + 1, :])
                    row = stage[:]
                sb = wpool.tile([128, w], f32, tag="sbcast")
                nc.gpsimd.partition_broadcast(sb[:], row)
                return sb

            def rope(ps, pool):
                """psum [128,TQ] f32 -> rope'd bf16 sbuf tile."""
                raw = pool.tile([128, TQ], bf16, tag="rraw", bufs=3, name="rraw")
                nc.scalar.copy(raw[:], ps[:])
                sw = pool.tile([128, TQ], bf16, tag="rsw", bufs=2, name="rsw")
                nc.sync.dma_start(sw[0:64, :], raw[64:128, :])
                nc.sync.dma_start(sw[64:128, :], raw[0:64, :])
                rr = pool.tile([128, TQ], bf16, tag="rr", bufs=4, name="rr")
                nc.vector.tensor_tensor(rr[:], raw[:], csc_sb[:], Alu.mult)
                t2 = pool.tile([128, TQ], bf16, tag="rt2", bufs=2, name="rt2")
                nc.vector.tensor_tensor(t2[:], sw[:], css_sb[:], Alu.mult)
                nc.vector.tensor_tensor(rr[:], rr[:], t2[:], Alu.add)
                return rr

            def sumsq(rr, pool):
                sq = pool.tile([128, TQ], bf16, tag="rsq", bufs=4, name="rsq")
                nc.vector.tensor_tensor(sq[:], rr[:], rr[:], Alu.mult)
                return sq

            with tc.tile_pool(name="attn", bufs=1) as ap_:
                mask_sb = ap_.tile([128, NT, TQ], bf16, tag="mask_sb")
                for i in range(NT):
                    nc.sync.dma_start(mask_sb[:, i],
                                      mask[128 * i:128 * (i + 1), :])
                qs_sb = ap_.tile([128, NH, TQ], bf16, tag="qs_sb")
                vloc = wsp.tile([128, 4, TQ], bf16, tag="vloc", bufs=1)

                with tc.tile_pool(name="ps1", bufs=1, space="PSUM") as ps1:
                    # ---- K heads first: project raw x + rope + k-norm ----
                    kps = [ps1.tile([128, TQ], f32, tag="qkv", bufs=4,
                                    name=f"kps_{_k}") for _k in range(4)]
                    wk_sb = wsp.tile([128, NT, TQ], bf16, tag="wslab", bufs=2,
                                     name="wk_sb")
                    for i in range(NT):
                        nc.sync.dma_start(wk_sb[:, i], wk[:, TQ * i:TQ * (i + 1)])
                    for k in range(4):
                        for i in range(NT):
                            nc.tensor.matmul(kps[k][:],
                                             lhsT=wk_sb[:, i, 128 * k:128 * (k + 1)],
                                             rhs=xin[:, i],
                                             start=(i == 0), stop=(i == NT - 1))
                    krr = []
                    ksps = ps1.tile([97, TQ], f32, tag="row4", bufs=2)
                    for kh in range(4):
                        rr = rope(kps[kh], ap_)
                        krr.append(rr)
                        sq = sumsq(rr, ap_)
                        nc.tensor.matmul(ksps[:], lhsT=e4[:, kh, :], rhs=sq[:],
                                         start=(kh == 0), stop=(kh == 3))
                    krows = rsqrt_rows(ksps, 1.0 / HD, EPS, 97)
                    for kh in range(4):
                        sb = bcast_row(krows, kh)
                        kt = ap_.tile([128, TQ], bf16, tag="ktile", bufs=3, name="kt")
                        nc.vector.tensor_tensor(kt[:], krr[kh][:], sb[:], Alu.mult)
                        nc.sync.dma_start(ck_in[128 * kh:128 * (kh + 1), :], kt[:])

                    nc.gpsimd.collective_compute(
                        "AllGather", Alu.bypass,
                        replica_groups=[[0, 1, 2, 3], [4, 5, 6, 7]],
                        ins=[ck_in[:]], outs=[ck_out[:]])

                    # ---- pre-norm sum-of-squares (only V needs the scale) ----
                    ssq_ps = ps1.tile([1, TQ], f32, tag="row", bufs=1)
                    for i in range(NT):
                        xsq = wpool.tile([128, TQ], bf16, tag="xsq", bufs=6)
                        nc.vector.tensor_tensor(xsq[:], xin[:, i], xin[:, i],
                                                Alu.mult)
                        nc.tensor.matmul(ssq_ps[:], lhsT=ones[:], rhs=xsq[:],
                                         start=(i == 0), stop=(i == NT - 1))
                    s1rows = rsqrt_rows(ssq_ps, 1.0 / C, EPS, 1)
                    s1b = bcast_row(s1rows, 0)

                    # ---- V heads: project + scale + transpose to token-major ----
                    vps = [ps1.tile([128, TQ], f32, tag="qkv", bufs=4,
                                    name=f"vps_{_k}") for _k in range(4)]
                    wv_sb = wsp.tile([128, NT, TQ], bf16, tag="wslab", bufs=2,
                                     name="wv_sb")
                    nc.sync.dma_start(wv_sb[:], wv.rearrange("p (g t) -> p g t", t=TQ))
                    for k in range(4):
                        for i in range(NT):
                            nc.tensor.matmul(vps[k][:],
                                             lhsT=wv_sb[:, i, 128 * k:128 * (k + 1)],
                                             rhs=xin[:, i],
                                             start=(i == 0), stop=(i == NT - 1))
                    for kh in range(4):
                        vb = ap_.tile([128, TQ], bf16, tag="ktile", bufs=3, name="vb")
                        nc.vector.tensor_tensor(vb[:], vps[kh][:], s1b[:],
                                                Alu.mult)
                        for j in range(4):
                            tps = ps1.tile([128, 128], bf16, tag="tr", bufs=1)
                            nc.tensor.transpose(tps[:], vb[:, 128 * j:128 * (j + 1)],
                                                ident[:])
                            nc.vector.tensor_copy(
                                out=vloc[:, j, 128 * kh:128 * (kh + 1)], in_=tps[:])
                    for j in range(4):
                        nc.sync.dma_start(
                            cv_in[128 * j:128 * (j + 1), :], vloc[:, j, :])
                    nc.gpsimd.collective_compute(
                        "AllGather", Alu.bypass,
                        replica_groups=[[0, 1, 2, 3], [4, 5, 6, 7]],
                        ins=[cv_in[:]], outs=[cv_out[:]])

                    # ---- Q heads: project + rope + deferred q-norm ----
                    # ssq matmuls of group g are emitted after group g+1's
                    # projection matmuls so the PE never waits on the DVE chain
                    pending = None

                    def finish_q(pend):
                        hg, rrs, sqs = pend
                        qsps = ps1.tile([97, TQ], f32, tag="row4", bufs=2)
                        for k in range(4):
                            nc.tensor.matmul(qsps[:], lhsT=e4[:, k, :],
                                             rhs=sqs[k][:],
                                             start=(k == 0), stop=(k == 3))
                        qrows = rsqrt_rows(qsps, 1.0, HD * EPS, 97)
                        for k in range(4):
                            h = 4 * hg + k
                            sb = bcast_row(qrows, k)
                            nc.vector.tensor_tensor(qs_sb[:, h], rrs[k][:], sb[:],
                                                    Alu.mult)

                    for hg in range(4):
                        qps = [ps1.tile([128, TQ], f32, tag="qkv", bufs=4,
                                        name=f"qps{hg}_{_k}") for _k in range(4)]
                        wq_sb = wsp.tile([128, NT, TQ], bf16, tag="wslab", bufs=2,
                                         name=f"wq_sb{hg}")
                        nc.sync.dma_start(
                            wq_sb[:],
                            wq[:, NT * TQ * hg:NT * TQ * (hg + 1)].rearrange(
                                "p (g t) -> p g t", t=TQ))
                        rrs = []
                        sqs = []
                        for k in range(4):
                            for i in range(NT):
                                nc.tensor.matmul(qps[k][:],
                                                 lhsT=wq_sb[:, i, 128 * k:128 * (k + 1)],
                                                 rhs=xin[:, i],
                                                 start=(i == 0), stop=(i == NT - 1))
                            rrs.append(rope(qps[k], ap_))
                            sqs.append(sumsq(rrs[k], ap_))
                        if pending is not None:
                            finish_q(pending)
                        pending = (hg, rrs, sqs)
                    finish_q(pending)

                # ---- load gathered K/V ----
                k_sb = ap_.tile([128, 16, TQ], bf16, tag="k_sb")   # (kh, g)
                v_sb = ap_.tile([128, 16, TQ], bf16, tag="v_sb")   # (g, j)
                for g in range(4):
                    for kh in range(4):
                        nc.sync.dma_start(
                            k_sb[:, 4 * kh + g],
                            ck_out[512 * g + 128 * kh:512 * g + 128 * (kh + 1), :])
                    for j in range(4):
                        nc.sync.dma_start(
                            v_sb[:, 4 * g + j],
                            cv_out[512 * g + 128 * j:512 * g + 128 * (j + 1), :])

                # ---- attention, 2 sibling q-heads per (kv head, pair) ----
                yT = ap_.tile([128, NH, TQ], bf16, tag="yT")
                with tc.tile_pool(name="ps2", bufs=1, space="PSUM") as ps2:
                    pending_epi = None

                    def emit_epi(epi):
                        hs, den2, y_ps = epi
                        dr = wpool.tile([33, TQ], f32, tag="srow", bufs=3,
                                        name="edr")
                        nc.vector.reciprocal(dr[:], den2[:])
                        for k in range(2):
                            db = bcast_row(dr, k)
                            nc.vector.tensor_tensor(yT[:, hs[k]], y_ps[k][:],
                                                    db[:], Alu.mult)

                    for kh in range(NKV):
                      for pr in range(2):
                        hs = [4 * kh + 2 * pr + k for k in range(2)]
                        den2 = ps2.tile([33, TQ], f32, tag="den", bufs=2)
                        y_ps = [ps2.tile([128, TQ], f32, tag="y", bufs=3,
                                         name=f"y{kh}{pr}_{_k}") for _k in range(2)]
                        fifo = []

                        def drain_one():
                            m0, k0, p0 = fifo.pop(0)
                            nc.tensor.matmul(den2[:],
                                             lhsT=e2[:, k0, :], rhs=p0[:],
                                             start=(m0 == 0 and k0 == 0),
                                             stop=(m0 == 15 and k0 == 1))
                            nc.tensor.matmul(
                                y_ps[k0][:],
                                lhsT=v_sb[:, m0, 128 * kh:128 * (kh + 1)],
                                rhs=p0[:],
                                start=(m0 == 0), stop=(m0 == 15))

                        for m in range(16):
                            g, mm = divmod(m, 4)
                            for k in range(2):
                                sc_ps = ps2.tile([128, TQ], f32, tag="sc", bufs=3)
                                nc.tensor.matmul(
                                    sc_ps[:],
                                    lhsT=k_sb[:, 4 * kh + g, 128 * mm:128 * (mm + 1)],
                                    rhs=qs_sb[:, hs[k]], start=True, stop=True)
                                p_bf = ap_.tile([128, TQ], bf16, tag="p_bf",
                                                bufs=6, name="p_bf")
                                nc.scalar.activation(p_bf[:], sc_ps[:], Act.Exp)
                                nc.vector.tensor_tensor(p_bf[:], p_bf[:],
                                                        mask_sb[:, m], Alu.mult)
                                fifo.append((m, k, p_bf))
                                if len(fifo) > 5:
                                    drain_one()
                            if m == 1 and pending_epi is not None:
                                emit_epi(pending_epi)
                                pending_epi = None
                        while fifo:
                            drain_one()
                        pending_epi = (hs, den2, y_ps)
                    emit_epi(pending_epi)

                # ---- wo projection + residual (feature-major xmT) ----
                with tc.tile_pool(name="ps2b", bufs=1, space="PSUM") as ps2b:
                    for n4 in range(4):
                        att_ps = [ps2b.tile([128, TQ], f32, tag="att", bufs=8,
                                            name=f"att{n4}_{_k}") for _k in range(4)]
                        wo_sb = wsp.tile([128, NT, TQ], bf16, tag="wslab", bufs=2,
                                         name=f"wo_sb{n4}")
                        nc.sync.dma_start(
                            wo_sb[:],
                            wo[:, NT * TQ * n4:NT * TQ * (n4 + 1)].rearrange(
                                "p (g t) -> p g t", t=TQ))
                        for k in range(4):
                            for h in range(NH):
                                nc.tensor.matmul(att_ps[k][:],
                                                 lhsT=wo_sb[:, h, 128 * k:128 * (k + 1)],
                                                 rhs=yT[:, h],
                                                 start=(h == 0), stop=(h == NH - 1))
                            n = 4 * n4 + k
                            nc.vector.tensor_tensor(xmT[:, n], att_ps[k][:],
                                                    xin[:, n], Alu.add)
            # attn pool closed

            # ---- MLP ----
            with tc.tile_pool(name="mlp", bufs=1) as mp:
                h2T = mp.tile([128, NT, TQ], bf16, tag="h2T")
                a_sb = mp.tile([128, NF, TQ], bf16, tag="a_sb")

                with tc.tile_pool(name="ps3", bufs=1, space="PSUM") as ps3:
                    ssq2 = ps3.tile([1, TQ], f32, tag="row", bufs=2)
                    for i in range(NT):
                        xsq = wpool.tile([128, TQ], bf16, tag="xsq", bufs=6)
                        nc.vector.tensor_tensor(xsq[:], xmT[:, i], xmT[:, i],
                                                Alu.mult)
                        nc.tensor.matmul(ssq2[:], lhsT=ones[:], rhs=xsq[:],
                                         start=(i == 0), stop=(i == NT - 1))
                    s2rows = rsqrt_rows(ssq2, 1.0 / C, EPS, 1)
                    s2b = bcast_row(s2rows, 0)
                    for i in range(NT):
                        nc.vector.tensor_tensor(h2T[:, i], xmT[:, i], s2b[:],
                                                Alu.mult)

                # fc + relu^2 (feature-major a)
                with tc.tile_pool(name="ps3b", bufs=1, space="PSUM") as ps3b:
                    for jc in range(16):
                        f_ps = [ps3b.tile([128, TQ], f32, tag="mm", bufs=8,
                                          name=f"fps{jc}_{_k}") for _k in range(4)]
                        wfc_sb = wsp.tile([128, NT, TQ], bf16, tag="wslab", bufs=2,
                                          name=f"wfc_sb{jc}")
                        nc.sync.dma_start(
                            wfc_sb[:],
                            wfc[:, NT * TQ * jc:NT * TQ * (jc + 1)].rearrange(
                                "p (g t) -> p g t", t=TQ))
                        for jf in range(4):
                            for i in range(NT):
                                nc.tensor.matmul(
                                    f_ps[jf][:],
                                    lhsT=wfc_sb[:, i, 128 * jf:128 * (jf + 1)],
                                    rhs=h2T[:, i],
                                    start=(i == 0), stop=(i == NT - 1))
                            f = 4 * jc + jf
                            r_bf = wpool.tile([128, TQ], bf16, tag="r_bf")
                            nc.scalar.activation(r_bf[:], f_ps[jf][:], Act.Relu)
                            nc.vector.tensor_tensor(a_sb[:, f], r_bf[:], r_bf[:],
                                                    Alu.mult)

                # proj: weight-stationary, feature-major output + residual
                with tc.tile_pool(name="ps4", bufs=1, space="PSUM") as ps4:
                    for cg in range(4):
                        o_ps = [ps4.tile([128, TQ], f32, tag="o", bufs=8,
                                         name=f"ops{cg}_{_k}") for _k in range(4)]
                        for f8 in range(4):
                            wp_sb = wsp.tile([128, 16, TQ], bf16, tag="wslab",
                                             bufs=2, name=f"wp{cg}_{f8}")
                            base = (cg * 4 + f8) * 16 * TQ
                            nc.sync.dma_start(
                                wp_sb[:],
                                wproj[:, base:base + 16 * TQ].rearrange(
                                    "p (g t) -> p g t", t=TQ))
                            for fl in range(16):
                                f = 16 * f8 + fl
                                for cl in range(4):
                                    nc.tensor.matmul(
                                        o_ps[cl][:],
                                        lhsT=wp_sb[:, fl, 128 * cl:128 * (cl + 1)],
                                        rhs=a_sb[:, f],
                                        start=(f == 0), stop=(f == NF - 1))
                        for cl in range(4):
                            c = 4 * cg + cl
                            ov = wpool.tile([128, TQ], f32, tag="sbcast")
                            nc.vector.tensor_tensor(ov[:], o_ps[cl][:],
                                                    xmT[:, c], Alu.add)
                            nc.sync.dma_start(
                                out_fm[128 * c:128 * (c + 1), :], ov[:])

    nc.compile()
    return nc


def _make_in_maps(x, cos, sin, weights_b):
    import ml_dtypes
    bf = ml_dtypes.bfloat16
    cosT = cos[0, :, 0, :].T  # [64, T]
    sinT = sin[0, :, 0, :].T
    cscf = np.concatenate([cosT, cosT], axis=0)   # [128, T]
    cssf = np.concatenate([sinT, -sinT], axis=0)
    in_maps = []
    for c in range(NCORES):
        b, r = divmod(c, 4)
        sl = slice(TQ * r, TQ * (r + 1))
        qpos = np.arange(TQ * r, TQ * (r + 1))
        m = {
            "xT": np.ascontiguousarray(x[b, sl, :].T).astype(bf),
            "csc": np.ascontiguousarray(cscf[:, sl]).astype(bf),
            "css": np.ascontiguousarray(cssf[:, sl]).astype(bf),
            "mask": (np.arange(T)[:, None] <= qpos[None, :]).astype(bf),
        }
        m.update(weights_b)
        in_maps.append(m)
    return in_maps


def _prep_weights(wq, wk, wv, wo, w_fc, w_proj):
    import ml_dtypes
    bf = ml_dtypes.bfloat16

    def tile_w(w, chunk):
        # [R, F] -> [128, (F//chunk) * (R//128) * chunk]: per output column
        # chunk, row-blocks become contiguous along the free axis
        R, F = w.shape
        t = w.reshape(R // 128, 128, F // chunk, chunk)
        t = t.transpose(1, 2, 0, 3)  # [128, F//chunk, R//128, chunk]
        return np.ascontiguousarray(t.reshape(128, -1)).astype(bf)

    wproj_f = np.asarray(w_proj, np.float32)
    # proj layout: slabs of (cg, f8): [p, cg, f8, f_local 16, c_local 4, 128]
    t = wproj_f.reshape(4, 16, 128, 4, 4, 128)  # [f8, fl, p, cg, cl, col]
    t = t.transpose(2, 3, 0, 1, 4, 5)           # [p, cg, f8, fl, cl, col]
    wproj_t = np.ascontiguousarray(t.reshape(128, -1)).astype(bf)

    return {
        "wq": tile_w(np.asarray(wq, np.float32), TQ),
        "wk": tile_w(np.asarray(wk, np.float32), NKV * HD),
        "wv": tile_w(np.asarray(wv, np.float32), NKV * HD),
        "wo": tile_w(np.asarray(wo, np.float32), TQ),
        "wfc": tile_w(np.asarray(w_fc, np.float32), TQ),
        "wproj": wproj_t,
    }


def kernel(x, cos, sin, wq, wk, wv, wo, w_fc, w_proj):
    global _CACHE
    from concourse.bass_utils import run_bass_kernel_spmd

    x = np.asarray(x, np.float32)
    cos = np.asarray(cos, np.float32)
    sin = np.asarray(sin, np.float32)
    weights_b = _prep_weights(wq, wk, wv, wo, w_fc, w_proj)

    if _CACHE is None:
        _CACHE = _build()
    nc = _CACHE

    in_maps = _make_in_maps(x, cos, sin, weights_b)
    res = run_bass_kernel_spmd(nc, in_maps, list(range(NCORES)))
    out = np.empty((B, T, C), np.float32)
    for c in range(NCORES):
        b, r = divmod(c, 4)
        out[b, TQ * r:TQ * (r + 1), :] = res.results[c]["out"].T
    return out


# revision 16
# speedup vs baseline: 1.2275x; 1.1001x over previous
"""Trainium2 Bass kernel for nn_Block_32762010534337 (dense transformer block).

Strategy: sequence-parallel over 8 cores. Core c owns 512 tokens (batch c//4,
token chunk c%4). Each core projects K/V from raw x (the pre-norm rmsnorm scale
commutes through the linear projections: rotary is per-token linear and the
q/k rmsnorms are scale-invariant, so only V needs the explicit scale),
AllGathers K/V within its batch group of 4 cores (overlapped with the Q
projections), then runs causal attention + wo + MLP (relu^2) for its 512
tokens with fully replicated bf16 weights. All rsqrt/reciprocal are computed
on the scalar engine as Exp(-a*Ln(x)) row ops to keep the DVE free.
Activations stay feature-major throughout; the host transposes per-core
inputs and the final output.
"""
import sys
import os

if "/opt/trn_rl_repo" not in sys.path:
    sys.path.insert(0, "/opt/trn_rl_repo")

import numpy as np

B, T, C = 2, 2048, 2048
NH, NKV, HD = 16, 4, 128
DFF = 4 * C
TQ = 512          # tokens per core
HQ = 256          # zig-zag half-chunk (front/back) query width
NT = C // 128     # 16 feature tiles
NF = DFF // 128   # 64 ff tiles
EPS = 1.1920929e-07
NCORES = 8

_CACHE = None


def _build():
    import concourse.bass as bass
    import concourse.tile as tile
    from concourse import mybir, bacc
    from concourse.masks import make_identity

    dt = mybir.dt
    f32, bf16 = dt.float32, dt.bfloat16
    Alu = mybir.AluOpType
    Act = mybir.ActivationFunctionType

    nc = bacc.Bacc("TRN2", target_bir_lowering=False, debug=False, num_devices=NCORES)

    for val in (0.0, EPS, HD * EPS):
        tns = nc.alloc_sbuf_tensor(f"const-f32-{val}", [128, 1], f32)
        nc.gpsimd.memset(tns.ap(), val)
        nc.const_aps.aps[(f32, val)] = tns.ap()
    nc.all_engine_barrier()

    xT = nc.declare_dram_parameter("xT", [C, TQ], bf16, isOutput=False)
    csc = nc.declare_dram_parameter("csc", [128, TQ], bf16, isOutput=False)
    css = nc.declare_dram_parameter("css", [128, TQ], bf16, isOutput=False)
    mask = nc.declare_dram_parameter("mask", [24 * 128, HQ], bf16, isOutput=False)
    # all weights host-pretiled to [128, n_tiles*512] (16KB-contiguous rows)
    wq = nc.declare_dram_parameter("wq", [128, 4 * NT * TQ], bf16, isOutput=False)
    wk = nc.declare_dram_parameter("wk", [128, NT * TQ], bf16, isOutput=False)
    wv = nc.declare_dram_parameter("wv", [128, NT * TQ], bf16, isOutput=False)
    wo = nc.declare_dram_parameter("wo", [128, 4 * NT * TQ], bf16, isOutput=False)
    wfc = nc.declare_dram_parameter("wfc", [128, 16 * NT * TQ], bf16, isOutput=False)
    wproj = nc.declare_dram_parameter("wproj", [128, 16 * NF * 128], bf16,
                                      isOutput=False)
    out_fm = nc.declare_dram_parameter("out", [C, TQ], f32, isOutput=True)

    ck_in = nc.dram_tensor("ck_in", [512, TQ], bf16)
    ck_out = nc.dram_tensor("ck_out", [2048, TQ], bf16)
    cv_in = nc.dram_tensor("cv_in", [512, TQ], bf16)
    cv_out = nc.dram_tensor("cv_out", [2048, TQ], bf16)

    with tile.TileContext(nc, num_cores=NCORES) as tc:
        with (
            tc.tile_pool(name="const", bufs=1) as constp,
            tc.tile_pool(name="persist", bufs=1) as pp,
            tc.tile_pool(name="work", bufs=3) as wpool,
            tc.tile_pool(name="wstream", bufs=3) as wsp,
        ):
            ident = constp.tile([128, 128], bf16, tag="ident")
            make_identity(nc, ident)
            ones = constp.tile([128, 1], bf16, tag="ones")
            nc.gpsimd.memset(ones, 1.0)
            # one-hot column selectors: e4[:, j, :] is [128,97] with col 32j
            # ones; e2[:, k, :] is [128,33] with col 32k ones. Used to land
            # ssq rows of different heads on 32-aligned PSUM partitions
            # (single-partition access must be 32-aligned) so one batched DVE
            # reciprocal covers a whole group of rows.
            e4 = constp.tile([128, 4, 97], bf16, tag="e4")
            nc.gpsimd.memset(e4, 0.0)
            for j in range(4):
                nc.gpsimd.memset(e4[:, j, 32 * j:32 * j + 1], 1.0)
            e2 = constp.tile([128, 2, 33], bf16, tag="e2")
            nc.gpsimd.memset(e2, 0.0)
            for k in range(2):
                nc.gpsimd.memset(e2[:, k, 32 * k:32 * k + 1], 1.0)
            csc_sb = constp.tile([128, TQ], bf16, tag="csc")
            nc.sync.dma_start(csc_sb[:], csc[:])
            css_sb = constp.tile([128, TQ], bf16, tag="css")
            nc.sync.dma_start(css_sb[:], css[:])

            # x (bf16) resident across the whole kernel; x_mid^T for MLP
            xin = pp.tile([128, NT, TQ], bf16, tag="xin")
            for i in range(NT):
                nc.sync.dma_start(xin[:, i], xT[128 * i:128 * (i + 1), :])
            xmT = pp.tile([128, NT, TQ], bf16, tag="xmT")

            def rsqrt_rows(ssq_ps, scale, bias, nrows):
                """[nrows,TQ] psum sum-of-squares -> [nrows,TQ] f32 sbuf
                1/sqrt(scale*x+bias): one Sqrt + one batched DVE recip."""
                sr = wpool.tile([nrows, TQ], f32, tag="srow", bufs=3,
                                name="srow")
                nc.scalar.activation(sr[:], ssq_ps[:], Act.Sqrt, bias=bias,
                                     scale=scale)
                rr_ = wpool.tile([nrows, TQ], f32, tag="srow", bufs=3,
                                 name="srow2")
                nc.vector.reciprocal(rr_[:], sr[:])
                return rr_

            def bcast_row(rows, j, w=TQ):
                # partition_broadcast reads partition 0 only: stage non-zero
                # rows down to a partition-0 tile via SBUF->SBUF DMA first.
                if j == 0:
                    row = rows[0:1, :]
                else:
                    stage = wpool.tile([1, w], f32, tag="stage", bufs=2,
                                       name="stage")
                    nc.sync.dma_start(stage[:], rows[32 * j:32 * j + 1, :])
                    row = stage[:]
                sb = wpool.tile([128, w], f32, tag="sbcast")
                nc.gpsimd.partition_broadcast(sb[:], row)
                return sb

            def rope(ps, pool):
                """psum [128,TQ] f32 -> rope'd bf16 sbuf tile."""
                raw = pool.tile([128, TQ], bf16, tag="rraw", bufs=3, name="rraw")
                nc.scalar.copy(raw[:], ps[:])
                sw = pool.tile([128, TQ], bf16, tag="rsw", bufs=2, name="rsw")
                nc.sync.dma_start(sw[0:64, :], raw[64:128, :])
                nc.sync.dma_start(sw[64:128, :], raw[0:64, :])
                rr = pool.tile([128, TQ], bf16, tag="rr", bufs=4, name="rr")
                nc.vector.tensor_tensor(rr[:], raw[:], csc_sb[:], Alu.mult)
                t2 = pool.tile([128, TQ], bf16, tag="rt2", bufs=2, name="rt2")
                nc.vector.tensor_tensor(t2[:], sw[:], css_sb[:], Alu.mult)
                nc.vector.tensor_tensor(rr[:], rr[:], t2[:], Alu.add)
                return rr

            def sumsq(rr, pool):
                sq = pool.tile([128, TQ], bf16, tag="rsq", bufs=4, name="rsq")
                nc.vector.tensor_tensor(sq[:], rr[:], rr[:], Alu.mult)
                return sq

            with tc.tile_pool(name="attn", bufs=1) as ap_:
                mask_sb = ap_.tile([128, 12, 2, HQ], bf16, tag="mask_sb")
                for p in range(12):
                    nc.sync.dma_start(
                        mask_sb[:, p],
                        mask[256 * p:256 * (p + 1), :].rearrange(
                            "(a p) t -> p a t", p=128))
                qs_sb = ap_.tile([128, NH, TQ], bf16, tag="qs_sb")
                vloc = wsp.tile([128, 4, TQ], bf16, tag="vloc", bufs=1)

                with tc.tile_pool(name="ps1", bufs=1, space="PSUM") as ps1:
                    # ---- K heads first: project raw x + rope + k-norm ----
                    kps = [ps1.tile([128, TQ], f32, tag="qkv", bufs=4,
                                    name=f"kps_{_k}") for _k in range(4)]
                    wk_sb = wsp.tile([128, NT, TQ], bf16, tag="wslab", bufs=2,
                                     name="wk_sb")
                    for i in range(NT):
                        nc.sync.dma_start(wk_sb[:, i], wk[:, TQ * i:TQ * (i + 1)])
                    for k in range(4):
                        for i in range(NT):
                            nc.tensor.matmul(kps[k][:],
                                             lhsT=wk_sb[:, i, 128 * k:128 * (k + 1)],
                                             rhs=xin[:, i],
                                             start=(i == 0), stop=(i == NT - 1))
                    krr = []
                    ksps = ps1.tile([97, TQ], f32, tag="row4", bufs=2)
                    for kh in range(4):
                        rr = rope(kps[kh], ap_)
                        krr.append(rr)
                        sq = sumsq(rr, ap_)
                        nc.tensor.matmul(ksps[:], lhsT=e4[:, kh, :], rhs=sq[:],
                                         start=(kh == 0), stop=(kh == 3))
                    krows = rsqrt_rows(ksps, 1.0 / HD, EPS, 97)
                    for kh in range(4):
                        sb = bcast_row(krows, kh)
                        kt = ap_.tile([128, TQ], bf16, tag="ktile", bufs=3, name="kt")
                        nc.vector.tensor_tensor(kt[:], krr[kh][:], sb[:], Alu.mult)
                        nc.sync.dma_start(ck_in[128 * kh:128 * (kh + 1), :], kt[:])

                    nc.gpsimd.collective_compute(
                        "AllGather", Alu.bypass,
                        replica_groups=[[0, 1, 2, 3], [4, 5, 6, 7]],
                        ins=[ck_in[:]], outs=[ck_out[:]])

                    # ---- pre-norm sum-of-squares (only V needs the scale) ----
                    ssq_ps = ps1.tile([1, TQ], f32, tag="row", bufs=1)
                    for i in range(NT):
                        xsq = wpool.tile([128, TQ], bf16, tag="xsq", bufs=6)
                        nc.vector.tensor_tensor(xsq[:], xin[:, i], xin[:, i],
                                                Alu.mult)
                        nc.tensor.matmul(ssq_ps[:], lhsT=ones[:], rhs=xsq[:],
                                         start=(i == 0), stop=(i == NT - 1))
                    s1rows = rsqrt_rows(ssq_ps, 1.0 / C, EPS, 1)
                    s1b = bcast_row(s1rows, 0)

                    # ---- V heads: project + scale + transpose to token-major ----
                    vps = [ps1.tile([128, TQ], f32, tag="qkv", bufs=4,
                                    name=f"vps_{_k}") for _k in range(4)]
                    wv_sb = wsp.tile([128, NT, TQ], bf16, tag="wslab", bufs=2,
                                     name="wv_sb")
                    nc.sync.dma_start(wv_sb[:], wv.rearrange("p (g t) -> p g t", t=TQ))
                    for k in range(4):
                        for i in range(NT):
                            nc.tensor.matmul(vps[k][:],
                                             lhsT=wv_sb[:, i, 128 * k:128 * (k + 1)],
                                             rhs=xin[:, i],
                                             start=(i == 0), stop=(i == NT - 1))
                    for kh in range(4):
                        vb = ap_.tile([128, TQ], bf16, tag="ktile", bufs=3, name="vb")
                        nc.vector.tensor_tensor(vb[:], vps[kh][:], s1b[:],
                                                Alu.mult)
                        for j in range(4):
                            tps = ps1.tile([128, 128], bf16, tag="tr", bufs=1)
                            nc.tensor.transpose(tps[:], vb[:, 128 * j:128 * (j + 1)],
                                                ident[:])
                            nc.vector.tensor_copy(
                                out=vloc[:, j, 128 * kh:128 * (kh + 1)], in_=tps[:])
                    for j in range(4):
                        nc.sync.dma_start(
                            cv_in[128 * j:128 * (j + 1), :], vloc[:, j, :])
                    nc.gpsimd.collective_compute(
                        "AllGather", Alu.bypass,
                        replica_groups=[[0, 1, 2, 3], [4, 5, 6, 7]],
                        ins=[cv_in[:]], outs=[cv_out[:]])

                    # ---- Q heads: project + rope + deferred q-norm ----
                    # ssq matmuls of group g are emitted after group g+1's
                    # projection matmuls so the PE never waits on the DVE chain
                    pending = None

                    def finish_q(pend):
                        hg, rrs, sqs = pend
                        qsps = ps1.tile([97, TQ], f32, tag="row4", bufs=2)
                        for k in range(4):
                            nc.tensor.matmul(qsps[:], lhsT=e4[:, k, :],
                                             rhs=sqs[k][:],
                                             start=(k == 0), stop=(k == 3))
                        qrows = rsqrt_rows(qsps, 1.0, HD * EPS, 97)
                        for k in range(4):
                            h = 4 * hg + k
                            sb = bcast_row(qrows, k)
                            nc.vector.tensor_tensor(qs_sb[:, h], rrs[k][:], sb[:],
                                                    Alu.mult)

                    for hg in range(4):
                        qps = [ps1.tile([128, TQ], f32, tag="qkv", bufs=4,
                                        name=f"qps{hg}_{_k}") for _k in range(4)]
                        wq_sb = wsp.tile([128, NT, TQ], bf16, tag="wslab", bufs=2,
                                         name=f"wq_sb{hg}")
                        nc.sync.dma_start(
                            wq_sb[:],
                            wq[:, NT * TQ * hg:NT * TQ * (hg + 1)].rearrange(
                                "p (g t) -> p g t", t=TQ))
                        rrs = []
                        sqs = []
                        for k in range(4):
                            for i in range(NT):
                                nc.tensor.matmul(qps[k][:],
                                                 lhsT=wq_sb[:, i, 128 * k:128 * (k + 1)],
                                                 rhs=xin[:, i],
                                                 start=(i == 0), stop=(i == NT - 1))
                            rrs.append(rope(qps[k], ap_))
                            sqs.append(sumsq(rrs[k], ap_))
                        if pending is not None:
                            finish_q(pending)
                        pending = (hg, rrs, sqs)
                    finish_q(pending)

                # ---- load gathered K/V ----
                k_sb = ap_.tile([128, 16, TQ], bf16, tag="k_sb")   # (kh, g)
                v_sb = ap_.tile([128, 16, TQ], bf16, tag="v_sb")   # (g, j)
                for g in range(4):
                    for kh in range(4):
                        nc.sync.dma_start(
                            k_sb[:, 4 * kh + g],
                            ck_out[512 * g + 128 * kh:512 * g + 128 * (kh + 1), :])
                    for j in range(4):
                        nc.sync.dma_start(
                            v_sb[:, 4 * g + j],
                            cv_out[512 * g + 128 * j:512 * g + 128 * (j + 1), :])

                # ---- attention, 2 sibling q-heads per (kv head, pair) ----
                yT = ap_.tile([128, NH, TQ], bf16, tag="yT")
                with tc.tile_pool(name="ps2", bufs=1, space="PSUM") as ps2:
                    pending_epi = None

                    def kvmap(m):
                        # global key tile m -> (source core g, local 128-tok tile)
                        if m < 8:
                            return m // 2, m % 2
                        if m % 2 == 0:
                            return (14 - m) // 2, 2
                        return (15 - m) // 2, 3

                    def emit_epi(epi):
                        hs, den4, y2 = epi
                        dr = wpool.tile([97, HQ], f32, tag="srow", bufs=3,
                                        name="edr")
                        nc.vector.reciprocal(dr[:], den4[:])
                        for k in range(2):
                            for half in range(2):
                                db = bcast_row(dr, 2 * k + half, w=HQ)
                                nc.vector.tensor_tensor(
                                    yT[:, hs[k], HQ * half:HQ * (half + 1)],
                                    y2[k][:, half, :], db[:], Alu.mult)

                    for kh in range(NKV):
                      for pr in range(2):
                        hs = [4 * kh + 2 * pr + k for k in range(2)]
                        den4 = ps2.tile([97, HQ], f32, tag="den", bufs=2)
                        y2 = [ps2.tile([128, 2, HQ], f32, tag="y", bufs=3,
                                       name=f"y{kh}{pr}_{_k}") for _k in range(2)]
                        fifo = []

                        def drain_one():
                            half0, s0, k0, pps0, p_bf0 = fifo.pop(0)
                            nc.tensor.matmul(
                                den4[:], lhsT=e4[:, 2 * k0 + half0, :],
                                rhs=pps0[:],
                                start=(half0 == 0 and s0 == 0 and k0 == 0),
                                stop=(half0 == 1 and s0 == 7 and k0 == 1))
                            for sl in range(2):
                                g, lt = kvmap(2 * s0 + sl)
                                nc.tensor.matmul(
                                    y2[k0][:, half0, :],
                                    lhsT=v_sb[:, 4 * g + lt,
                                              128 * kh:128 * (kh + 1)],
                                    rhs=p_bf0[:, sl, :],
                                    start=(half0 == 0 and s0 == 0 and sl == 0),
                                    stop=(half0 == 1 and s0 == 7 and sl == 1))

                        for half in range(2):
                            npairs = 4 if half == 0 else 8
                            for s in range(npairs):
                                for k in range(2):
                                    sc2 = ps2.tile([128, 2, HQ], f32, tag="sc",
                                                   bufs=3)
                                    for sl in range(2):
                                        g, lt = kvmap(2 * s + sl)
                                        nc.tensor.matmul(
                                            sc2[:, sl, :],
                                            lhsT=k_sb[:, 4 * kh + g,
                                                      128 * lt:128 * (lt + 1)],
                                            rhs=qs_sb[:, hs[k],
                                                      HQ * half:HQ * (half + 1)],
                                            start=(sl == 0), stop=(sl == 1))
                                    p_bf = ap_.tile([128, 2, HQ], bf16,
                                                    tag="p_bf", bufs=6,
                                                    name="p_bf")
                                    nc.scalar.activation(p_bf[:], sc2[:],
                                                         Act.Exp)
                                    P = s if half == 0 else 4 + s
                                    nc.vector.tensor_tensor(p_bf[:], p_bf[:],
                                                            mask_sb[:, P],
                                                            Alu.mult)
                                    pps = ap_.tile([128, HQ], bf16, tag="pps",
                                                   bufs=6, name="pps")
                                    nc.vector.tensor_tensor(pps[:],
                                                            p_bf[:, 0, :],
                                                            p_bf[:, 1, :],
                                                            Alu.add)
                                    fifo.append((half, s, k, pps, p_bf))
                                    if len(fifo) > 3:
                                        drain_one()
                                if half == 0 and s == 1 and pending_epi is not None:
                                    emit_epi(pending_epi)
                                    pending_epi = None
                        while fifo:
                            drain_one()
                        pending_epi = (hs, den4, y2)
                    emit_epi(pending_epi)

                # ---- wo projection + residual (feature-major xmT) ----
                with tc.tile_pool(name="ps2b", bufs=1, space="PSUM") as ps2b:
                    for n4 in range(4):
                        att_ps = [ps2b.tile([128, TQ], f32, tag="att", bufs=8,
                                            name=f"att{n4}_{_k}") for _k in range(4)]
                        wo_sb = wsp.tile([128, NT, TQ], bf16, tag="wslab", bufs=2,
                                         name=f"wo_sb{n4}")
                        nc.sync.dma_start(
                            wo_sb[:],
                            wo[:, NT * TQ * n4:NT * TQ * (n4 + 1)].rearrange(
                                "p (g t) -> p g t", t=TQ))
                        for k in range(4):
                            for h in range(NH):
                                nc.tensor.matmul(att_ps[k][:],
                                                 lhsT=wo_sb[:, h, 128 * k:128 * (k + 1)],
                                                 rhs=yT[:, h],
                                                 start=(h == 0), stop=(h == NH - 1))
                            n = 4 * n4 + k
                            nc.vector.tensor_tensor(xmT[:, n], att_ps[k][:],
                                                    xin[:, n], Alu.add)
            # attn pool closed

            # ---- MLP ----
            with tc.tile_pool(name="mlp", bufs=1) as mp:
                h2T = mp.tile([128, NT, TQ], bf16, tag="h2T")
                a_sb = mp.tile([128, NF, TQ], bf16, tag="a_sb")

                with tc.tile_pool(name="ps3", bufs=1, space="PSUM") as ps3:
                    ssq2 = ps3.tile([1, TQ], f32, tag="row", bufs=2)
                    for i in range(NT):
                        xsq = wpool.tile([128, TQ], bf16, tag="xsq", bufs=6)
                        nc.vector.tensor_tensor(xsq[:], xmT[:, i], xmT[:, i],
                                                Alu.mult)
                        nc.tensor.matmul(ssq2[:], lhsT=ones[:], rhs=xsq[:],
                                         start=(i == 0), stop=(i == NT - 1))
                    s2rows = rsqrt_rows(ssq2, 1.0 / C, EPS, 1)
                    s2b = bcast_row(s2rows, 0)
                    for i in range(NT):
                        nc.vector.tensor_tensor(h2T[:, i], xmT[:, i], s2b[:],
                                                Alu.mult)

                # fc + relu^2 (feature-major a)
                with tc.tile_pool(name="ps3b", bufs=1, space="PSUM") as ps3b:
                    for jc in range(16):
                        f_ps = [ps3b.tile([128, TQ], f32, tag="mm", bufs=8,
                                          name=f"fps{jc}_{_k}") for _k in range(4)]
                        wfc_sb = wsp.tile([128, NT, TQ], bf16, tag="wslab", bufs=2,
                                          name=f"wfc_sb{jc}")
                        nc.sync.dma_start(
                            wfc_sb[:],
                            wfc[:, NT * TQ * jc:NT * TQ * (jc + 1)].rearrange(
                                "p (g t) -> p g t", t=TQ))
                        for jf in range(4):
                            for i in range(NT):
                                nc.tensor.matmul(
                                    f_ps[jf][:],
                                    lhsT=wfc_sb[:, i, 128 * jf:128 * (jf + 1)],
                                    rhs=h2T[:, i],
                                    start=(i == 0), stop=(i == NT - 1))
                            f = 4 * jc + jf
                            r_bf = wpool.tile([128, TQ], bf16, tag="r_bf")
                            nc.scalar.activation(r_bf[:], f_ps[jf][:], Act.Relu)
                            nc.vector.tensor_tensor(a_sb[:, f], r_bf[:], r_bf[:],
                                                    Alu.mult)

                # proj: weight-stationary, feature-major output + residual
                with tc.tile_pool(name="ps4", bufs=1, space="PSUM") as ps4:
                    for cg in range(4):
                        o_ps = [ps4.tile([128, TQ], f32, tag="o", bufs=8,
                                         name=f"ops{cg}_{_k}") for _k in range(4)]
                        for f8 in range(4):
                            wp_sb = wsp.tile([128, 16, TQ], bf16, tag="wslab",
                                             bufs=2, name=f"wp{cg}_{f8}")
                            base = (cg * 4 + f8) * 16 * TQ
                            nc.sync.dma_start(
                                wp_sb[:],
                                wproj[:, base:base + 16 * TQ].rearrange(
                                    "p (g t) -> p g t", t=TQ))
                            for fl in range(16):
                                f = 16 * f8 + fl
                                for cl in range(4):
                                    nc.tensor.matmul(
                                        o_ps[cl][:],
                                        lhsT=wp_sb[:, fl, 128 * cl:128 * (cl + 1)],
                                        rhs=a_sb[:, f],
                                        start=(f == 0), stop=(f == NF - 1))
                        for cl in range(4):
                            c = 4 * cg + cl
                            ov = wpool.tile([128, TQ], f32, tag="sbcast")
                            nc.vector.tensor_tensor(ov[:], o_ps[cl][:],
                                                    xmT[:, c], Alu.add)
                            nc.sync.dma_start(
                                out_fm[128 * c:128 * (c + 1), :], ov[:])

    nc.compile()
    return nc


def _core_perm(r):
    # zig-zag: front 256 tokens + mirrored back 256 tokens
    return np.concatenate([np.arange(256 * r, 256 * (r + 1)),
                           np.arange(T - 256 * (r + 1), T - 256 * r)])


def _make_in_maps(x, cos, sin, weights_b):
    import ml_dtypes
    bf = ml_dtypes.bfloat16
    cosT = cos[0, :, 0, :].T  # [64, T]
    sinT = sin[0, :, 0, :].T
    cscf = np.concatenate([cosT, cosT], axis=0)   # [128, T]
    cssf = np.concatenate([sinT, -sinT], axis=0)
    in_maps = []
    for c in range(NCORES):
        b, r = divmod(c, 4)
        perm = _core_perm(r)
        qf, qb = perm[:HQ], perm[HQ:]
        mf = np.arange(8 * 128)[:, None] <= qf[None, :]    # front: keys 0..1023
        mb = np.arange(16 * 128)[:, None] <= qb[None, :]   # back: keys 0..2047
        m = {
            "xT": np.ascontiguousarray(x[b, perm, :].T).astype(bf),
            "csc": np.ascontiguousarray(cscf[:, perm]).astype(bf),
            "css": np.ascontiguousarray(cssf[:, perm]).astype(bf),
            "mask": np.concatenate([mf, mb], axis=0).astype(bf),
        }
        m.update(weights_b)
        in_maps.append(m)
    return in_maps


def _prep_weights(wq, wk, wv, wo, w_fc, w_proj):
    import ml_dtypes
    bf = ml_dtypes.bfloat16

    def tile_w(w, chunk):
        # [R, F] -> [128, (F//chunk) * (R//128) * chunk]: per output column
        # chunk, row-blocks become contiguous along the free axis
        R, F = w.shape
        t = w.reshape(R // 128, 128, F // chunk, chunk)
        t = t.transpose(1, 2, 0, 3)  # [128, F//chunk, R//128, chunk]
        return np.ascontiguousarray(t.reshape(128, -1)).astype(bf)

    wproj_f = np.asarray(w_proj, np.float32)
    # proj layout: slabs of (cg, f8): [p, cg, f8, f_local 16, c_local 4, 128]
    t = wproj_f.reshape(4, 16, 128, 4, 4, 128)  # [f8, fl, p, cg, cl, col]
    t = t.transpose(2, 3, 0, 1, 4, 5)           # [p, cg, f8, fl, cl, col]
    wproj_t = np.ascontiguousarray(t.reshape(128, -1)).astype(bf)

    return {
        "wq": tile_w(np.asarray(wq, np.float32), TQ),
        "wk": tile_w(np.asarray(wk, np.float32), NKV * HD),
        "wv": tile_w(np.asarray(wv, np.float32), NKV * HD),
        "wo": tile_w(np.asarray(wo, np.float32), TQ),
        "wfc": tile_w(np.asarray(w_fc, np.float32), TQ),
        "wproj": wproj_t,
    }


def kernel(x, cos, sin, wq, wk, wv, wo, w_fc, w_proj):
    global _CACHE
    from concourse.bass_utils import run_bass_kernel_spmd

    x = np.asarray(x, np.float32)
    cos = np.asarray(cos, np.float32)
    sin = np.asarray(sin, np.float32)
    weights_b = _prep_weights(wq, wk, wv, wo, w_fc, w_proj)

    if _CACHE is None:
        _CACHE = _build()
    nc = _CACHE

    in_maps = _make_in_maps(x, cos, sin, weights_b)
    res = run_bass_kernel_spmd(nc, in_maps, list(range(NCORES)))
    out = np.empty((B, T, C), np.float32)
    for c in range(NCORES):
        b, r = divmod(c, 4)
        out[b, _core_perm(r), :] = res.results[c]["out"].T
    return out


# revision 17
# speedup vs baseline: 1.2644x; 1.0301x over previous
"""Trainium2 Bass kernel for nn_Block_32762010534337 (dense transformer block).

Strategy: sequence-parallel over 8 cores. Core c owns 512 tokens (batch c//4,
token chunk c%4). Each core projects K/V from raw x (the pre-norm rmsnorm scale
commutes through the linear projections: rotary is per-token linear and the
q/k rmsnorms are scale-invariant, so only V needs the explicit scale),
AllGathers K/V within its batch group of 4 cores (overlapped with the Q
projections), then runs causal attention + wo + MLP (relu^2) for its 512
tokens with fully replicated bf16 weights. All rsqrt/reciprocal are computed
on the scalar engine as Exp(-a*Ln(x)) row ops to keep the DVE free.
Activations stay feature-major throughout; the host transposes per-core
inputs and the final output.
"""
import sys
import os

if "/opt/trn_rl_repo" not in sys.path:
    sys.path.insert(0, "/opt/trn_rl_repo")

import numpy as np

B, T, C = 2, 2048, 2048
NH, NKV, HD = 16, 4, 128
DFF = 4 * C
TQ = 512          # tokens per core
HQ = 256          # zig-zag half-chunk (front/back) query width
NT = C // 128     # 16 feature tiles
NF = DFF // 128   # 64 ff tiles
EPS = 1.1920929e-07
NCORES = 8

_CACHE = None


def _build():
    import concourse.bass as bass
    import concourse.tile as tile
    from concourse import mybir, bacc
    from concourse.masks import make_identity

    dt = mybir.dt
    f32, bf16 = dt.float32, dt.bfloat16
    Alu = mybir.AluOpType
    Act = mybir.ActivationFunctionType

    nc = bacc.Bacc("TRN2", target_bir_lowering=False, debug=False, num_devices=NCORES)

    for val in (0.0, EPS, HD * EPS):
        tns = nc.alloc_sbuf_tensor(f"const-f32-{val}", [128, 1], f32)
        nc.gpsimd.memset(tns.ap(), val)
        nc.const_aps.aps[(f32, val)] = tns.ap()
    nc.all_engine_barrier()

    xT = nc.declare_dram_parameter("xT", [C, TQ], bf16, isOutput=False)
    csc = nc.declare_dram_parameter("csc", [128, TQ], bf16, isOutput=False)
    css = nc.declare_dram_parameter("css", [128, TQ], bf16, isOutput=False)
    mask = nc.declare_dram_parameter("mask", [24 * 128, HQ], bf16, isOutput=False)
    # all weights host-pretiled to [128, n_tiles*512] (16KB-contiguous rows)
    wq = nc.declare_dram_parameter("wq", [128, 4 * NT * TQ], bf16, isOutput=False)
    wk = nc.declare_dram_parameter("wk", [128, NT * TQ], bf16, isOutput=False)
    wv = nc.declare_dram_parameter("wv", [128, NT * TQ], bf16, isOutput=False)
    wo = nc.declare_dram_parameter("wo", [128, 4 * NT * TQ], bf16, isOutput=False)
    wfc = nc.declare_dram_parameter("wfc", [128, 16 * NT * TQ], bf16, isOutput=False)
    wproj = nc.declare_dram_parameter("wproj", [128, 16 * NF * 128], bf16,
                                      isOutput=False)
    out_fm = nc.declare_dram_parameter("out", [C, TQ], f32, isOutput=True)

    ck_in = nc.dram_tensor("ck_in", [512, TQ], bf16)
    ck_out = nc.dram_tensor("ck_out", [2048, TQ], bf16)
    cv_in = nc.dram_tensor("cv_in", [512, TQ], bf16)
    cv_out = nc.dram_tensor("cv_out", [2048, TQ], bf16)

    with tile.TileContext(nc, num_cores=NCORES) as tc:
        with (
            tc.tile_pool(name="const", bufs=1) as constp,
            tc.tile_pool(name="persist", bufs=1) as pp,
            tc.tile_pool(name="work", bufs=3) as wpool,
            tc.tile_pool(name="wstream", bufs=3) as wsp,
        ):
            ident = constp.tile([128, 128], bf16, tag="ident")
            make_identity(nc, ident)
            ones = constp.tile([128, 1], bf16, tag="ones")
            nc.gpsimd.memset(ones, 1.0)
            # one-hot column selectors: e4[:, j, :] is [128,97] with col 32j
            # ones; e2[:, k, :] is [128,33] with col 32k ones. Used to land
            # ssq rows of different heads on 32-aligned PSUM partitions
            # (single-partition access must be 32-aligned) so one batched DVE
            # reciprocal covers a whole group of rows.
            e4 = constp.tile([128, 4, 97], bf16, tag="e4")
            nc.gpsimd.memset(e4, 0.0)
            for j in range(4):
                nc.gpsimd.memset(e4[:, j, 32 * j:32 * j + 1], 1.0)
            e2 = constp.tile([128, 2, 33], bf16, tag="e2")
            nc.gpsimd.memset(e2, 0.0)
            for k in range(2):
                nc.gpsimd.memset(e2[:, k, 32 * k:32 * k + 1], 1.0)
            csc_sb = constp.tile([128, TQ], bf16, tag="csc")
            nc.sync.dma_start(csc_sb[:], csc[:])
            css_sb = constp.tile([128, TQ], bf16, tag="css")
            nc.sync.dma_start(css_sb[:], css[:])

            # x (bf16) resident across the whole kernel; x_mid^T for MLP
            xin = pp.tile([128, NT, TQ], bf16, tag="xin")
            for i in range(NT):
                nc.sync.dma_start(xin[:, i], xT[128 * i:128 * (i + 1), :])
            xmT = pp.tile([128, NT, TQ], bf16, tag="xmT")

            def rsqrt_rows(ssq_ps, scale, bias, nrows):
                """[nrows,TQ] psum sum-of-squares -> [nrows,TQ] f32 sbuf
                1/sqrt(scale*x+bias): one Sqrt + one batched DVE recip."""
                sr = wpool.tile([nrows, TQ], f32, tag="srow", bufs=3,
                                name="srow")
                nc.scalar.activation(sr[:], ssq_ps[:], Act.Sqrt, bias=bias,
                                     scale=scale)
                rr_ = wpool.tile([nrows, TQ], f32, tag="srow", bufs=3,
                                 name="srow2")
                nc.vector.reciprocal(rr_[:], sr[:])
                return rr_

            def bcast_row(rows, j, w=TQ):
                # partition_broadcast reads partition 0 only: stage non-zero
                # rows down to a partition-0 tile via SBUF->SBUF DMA first.
                if j == 0:
                    row = rows[0:1, :]
                else:
                    stage = wpool.tile([1, w], f32, tag="stage", bufs=2,
                                       name="stage")
                    nc.sync.dma_start(stage[:], rows[32 * j:32 * j + 1, :])
                    row = stage[:]
                sb = wpool.tile([128, w], f32, tag="sbcast")
                nc.gpsimd.partition_broadcast(sb[:], row)
                return sb

            def rope(ps, pool):
                """psum [128,TQ] f32 -> rope'd bf16 sbuf tile."""
                raw = pool.tile([128, TQ], bf16, tag="rraw", bufs=3, name="rraw")
                nc.scalar.copy(raw[:], ps[:])
                sw = pool.tile([128, TQ], bf16, tag="rsw", bufs=2, name="rsw")
                nc.sync.dma_start(sw[0:64, :], raw[64:128, :])
                nc.sync.dma_start(sw[64:128, :], raw[0:64, :])
                rr = pool.tile([128, TQ], bf16, tag="rr", bufs=4, name="rr")
                nc.vector.tensor_tensor(rr[:], raw[:], csc_sb[:], Alu.mult)
                t2 = pool.tile([128, TQ], bf16, tag="rt2", bufs=2, name="rt2")
                nc.vector.tensor_tensor(t2[:], sw[:], css_sb[:], Alu.mult)
                nc.vector.tensor_tensor(rr[:], rr[:], t2[:], Alu.add)
                return rr

            def sumsq(rr, pool):
                sq = pool.tile([128, TQ], bf16, tag="rsq", bufs=4, name="rsq")
                nc.vector.tensor_tensor(sq[:], rr[:], rr[:], Alu.mult)
                return sq

            with tc.tile_pool(name="attn", bufs=1) as ap_:
                mask_sb = ap_.tile([128, 12, 2, HQ], bf16, tag="mask_sb")
                qs_sb = ap_.tile([128, NH, TQ], bf16, tag="qs_sb")
                vloc = wsp.tile([128, 4, TQ], bf16, tag="vloc", bufs=1)

                with tc.tile_pool(name="ps1", bufs=1, space="PSUM") as ps1:
                    # ---- K heads first: project raw x + rope + k-norm ----
                    kps = [ps1.tile([128, TQ], f32, tag="qkv", bufs=4,
                                    name=f"kps_{_k}") for _k in range(4)]
                    wk_sb = wsp.tile([128, NT, TQ], bf16, tag="wslab", bufs=2,
                                     name="wk_sb")
                    for i in range(NT):
                        nc.sync.dma_start(wk_sb[:, i], wk[:, TQ * i:TQ * (i + 1)])
                    for k in range(4):
                        for i in range(NT):
                            nc.tensor.matmul(kps[k][:],
                                             lhsT=wk_sb[:, i, 128 * k:128 * (k + 1)],
                                             rhs=xin[:, i],
                                             start=(i == 0), stop=(i == NT - 1))
                    # ---- pre-norm sum-of-squares (only V needs the scale,
                    # emitted early so s1 is ready when V finishes) ----
                    ssq_ps = ps1.tile([1, TQ], f32, tag="row", bufs=1)
                    for i in range(NT):
                        xsq = wpool.tile([128, TQ], bf16, tag="xsq", bufs=6)
                        nc.vector.tensor_tensor(xsq[:], xin[:, i], xin[:, i],
                                                Alu.mult)
                        nc.tensor.matmul(ssq_ps[:], lhsT=ones[:], rhs=xsq[:],
                                         start=(i == 0), stop=(i == NT - 1))
                    s1rows = rsqrt_rows(ssq_ps, 1.0 / C, EPS, 1)
                    s1b = bcast_row(s1rows, 0)

                    # ---- V heads: project raw x ----
                    vps = [ps1.tile([128, TQ], f32, tag="qkv", bufs=4,
                                    name=f"vps_{_k}") for _k in range(4)]
                    wv_sb = wsp.tile([128, NT, TQ], bf16, tag="wslab", bufs=2,
                                     name="wv_sb")
                    for i in range(NT):
                        nc.sync.dma_start(wv_sb[:, i], wv[:, TQ * i:TQ * (i + 1)])
                    for k in range(4):
                        for i in range(NT):
                            nc.tensor.matmul(vps[k][:],
                                             lhsT=wv_sb[:, i, 128 * k:128 * (k + 1)],
                                             rhs=xin[:, i],
                                             start=(i == 0), stop=(i == NT - 1))

                    # ---- K rope + k-norm -> ck_in ----
                    krr = []
                    ksps = ps1.tile([97, TQ], f32, tag="row4", bufs=2)
                    for kh in range(4):
                        rr = rope(kps[kh], ap_)
                        krr.append(rr)
                        sq = sumsq(rr, ap_)
                        nc.tensor.matmul(ksps[:], lhsT=e4[:, kh, :], rhs=sq[:],
                                         start=(kh == 0), stop=(kh == 3))
                    krows = rsqrt_rows(ksps, 1.0 / HD, EPS, 97)
                    for kh in range(4):
                        sb = bcast_row(krows, kh)
                        kt = ap_.tile([128, TQ], bf16, tag="ktile", bufs=3, name="kt")
                        nc.vector.tensor_tensor(kt[:], krr[kh][:], sb[:], Alu.mult)
                        nc.sync.dma_start(ck_in[128 * kh:128 * (kh + 1), :], kt[:])

                    # ---- V scale + transpose to token-major -> cv_in ----
                    for kh in range(4):
                        vb = ap_.tile([128, TQ], bf16, tag="ktile", bufs=3, name="vb")
                        nc.vector.tensor_tensor(vb[:], vps[kh][:], s1b[:],
                                                Alu.mult)
                        for j in range(4):
                            tps = ps1.tile([128, 128], bf16, tag="tr", bufs=1)
                            nc.tensor.transpose(tps[:], vb[:, 128 * j:128 * (j + 1)],
                                                ident[:])
                            nc.vector.tensor_copy(
                                out=vloc[:, j, 128 * kh:128 * (kh + 1)], in_=tps[:])
                    for j in range(4):
                        nc.sync.dma_start(
                            cv_in[128 * j:128 * (j + 1), :], vloc[:, j, :])

                    # ---- one merged K+V AllGather ----
                    nc.gpsimd.collective_compute(
                        "AllGather", Alu.bypass,
                        replica_groups=[[0, 1, 2, 3], [4, 5, 6, 7]],
                        ins=[ck_in[:], cv_in[:]], outs=[ck_out[:], cv_out[:]])

                    # ---- Q heads: project + rope + deferred q-norm ----
                    # ssq matmuls of group g are emitted after group g+1's
                    # projection matmuls so the PE never waits on the DVE chain
                    pending = None

                    def finish_q(pend):
                        hg, rrs, sqs = pend
                        qsps = ps1.tile([97, TQ], f32, tag="row4", bufs=2)
                        for k in range(4):
                            nc.tensor.matmul(qsps[:], lhsT=e4[:, k, :],
                                             rhs=sqs[k][:],
                                             start=(k == 0), stop=(k == 3))
                        qrows = rsqrt_rows(qsps, 1.0, HD * EPS, 97)
                        for k in range(4):
                            h = 4 * hg + k
                            sb = bcast_row(qrows, k)
                            nc.vector.tensor_tensor(qs_sb[:, h], rrs[k][:], sb[:],
                                                    Alu.mult)

                    for hg in range(4):
                        qps = [ps1.tile([128, TQ], f32, tag="qkv", bufs=4,
                                        name=f"qps{hg}_{_k}") for _k in range(4)]
                        wq_sb = wsp.tile([128, NT, TQ], bf16, tag="wslab", bufs=2,
                                         name=f"wq_sb{hg}")
                        nc.sync.dma_start(
                            wq_sb[:],
                            wq[:, NT * TQ * hg:NT * TQ * (hg + 1)].rearrange(
                                "p (g t) -> p g t", t=TQ))
                        rrs = []
                        sqs = []
                        for k in range(4):
                            for i in range(NT):
                                nc.tensor.matmul(qps[k][:],
                                                 lhsT=wq_sb[:, i, 128 * k:128 * (k + 1)],
                                                 rhs=xin[:, i],
                                                 start=(i == 0), stop=(i == NT - 1))
                            rrs.append(rope(qps[k], ap_))
                            sqs.append(sumsq(rrs[k], ap_))
                        if pending is not None:
                            finish_q(pending)
                        pending = (hg, rrs, sqs)
                    finish_q(pending)

                for p in range(12):
                    nc.sync.dma_start(
                        mask_sb[:, p],
                        mask[256 * p:256 * (p + 1), :].rearrange(
                            "(a p) t -> p a t", p=128))
                # ---- load gathered K/V ----
                k_sb = ap_.tile([128, 16, TQ], bf16, tag="k_sb")   # (kh, g)
                v_sb = ap_.tile([128, 16, TQ], bf16, tag="v_sb")   # (g, j)
                for g in range(4):
                    for kh in range(4):
                        nc.sync.dma_start(
                            k_sb[:, 4 * kh + g],
                            ck_out[512 * g + 128 * kh:512 * g + 128 * (kh + 1), :])
                    for j in range(4):
                        nc.sync.dma_start(
                            v_sb[:, 4 * g + j],
                            cv_out[512 * g + 128 * j:512 * g + 128 * (j + 1), :])

                # ---- attention, 2 sibling q-heads per (kv head, pair) ----
                yT = ap_.tile([128, NH, TQ], bf16, tag="yT")
                with tc.tile_pool(name="ps2", bufs=1, space="PSUM") as ps2:
                    pending_epi = None

                    def kvmap(m):
                        # global key tile m -> (source core g, local 128-tok tile)
                        if m < 8:
                            return m // 2, m % 2
                        if m % 2 == 0:
                            return (14 - m) // 2, 2
                        return (15 - m) // 2, 3

                    def emit_epi(epi):
                        hs, den4, y2 = epi
                        dr = wpool.tile([97, HQ], f32, tag="srow", bufs=3,
                                        name="edr")
                        nc.vector.reciprocal(dr[:], den4[:])
                        for k in range(2):
                            for half in range(2):
                                db = bcast_row(dr, 2 * k + half, w=HQ)
                                nc.vector.tensor_tensor(
                                    yT[:, hs[k], HQ * half:HQ * (half + 1)],
                                    y2[k][:, half, :], db[:], Alu.mult)

                    for kh in range(NKV):
                      for pr in range(2):
                        hs = [4 * kh + 2 * pr + k for k in range(2)]
                        den4 = ps2.tile([97, HQ], f32, tag="den", bufs=2)
                        y2 = [ps2.tile([128, 2, HQ], f32, tag="y", bufs=3,
                                       name=f"y{kh}{pr}_{_k}") for _k in range(2)]
                        fifo = []

                        def drain_one():
                            half0, s0, k0, pps0, p_bf0 = fifo.pop(0)
                            nc.tensor.matmul(
                                den4[:], lhsT=e4[:, 2 * k0 + half0, :],
                                rhs=pps0[:],
                                start=(half0 == 0 and s0 == 0 and k0 == 0),
                                stop=(half0 == 1 and s0 == 7 and k0 == 1))
                            for sl in range(2):
                                g, lt = kvmap(2 * s0 + sl)
                                nc.tensor.matmul(
                                    y2[k0][:, half0, :],
                                    lhsT=v_sb[:, 4 * g + lt,
                                              128 * kh:128 * (kh + 1)],
                                    rhs=p_bf0[:, sl, :],
                                    start=(half0 == 0 and s0 == 0 and sl == 0),
                                    stop=(half0 == 1 and s0 == 7 and sl == 1))

                        for half in range(2):
                            npairs = 4 if half == 0 else 8
                            for s in range(npairs):
                                for k in range(2):
                                    sc2 = ps2.tile([128, 2, HQ], f32, tag="sc",
                                                   bufs=3)
                                    for sl in range(2):
                                        g, lt = kvmap(2 * s + sl)
                                        nc.tensor.matmul(
                                            sc2[:, sl, :],
                                            lhsT=k_sb[:, 4 * kh + g,
                                                      128 * lt:128 * (lt + 1)],
                                            rhs=qs_sb[:, hs[k],
                                                      HQ * half:HQ * (half + 1)],
                                            start=(sl == 0), stop=(sl == 1))
                                    p_bf = ap_.tile([128, 2, HQ], bf16,
                                                    tag="p_bf", bufs=6,
                                                    name="p_bf")
                                    nc.scalar.activation(p_bf[:], sc2[:],
                                                         Act.Exp)
                                    P = s if half == 0 else 4 + s
                                    nc.vector.tensor_tensor(p_bf[:], p_bf[:],
                                                            mask_sb[:, P],
                                                            Alu.mult)
                                    pps = ap_.tile([128, HQ], bf16, tag="pps",
                                                   bufs=6, name="pps")
                                    nc.vector.tensor_tensor(pps[:],
                                                            p_bf[:, 0, :],
                                                            p_bf[:, 1, :],
                                                            Alu.add)
                                    fifo.append((half, s, k, pps, p_bf))
                                    if len(fifo) > 3:
                                        drain_one()
                                if half == 0 and s == 1 and pending_epi is not None:
                                    emit_epi(pending_epi)
                                    pending_epi = None
                        while fifo:
                            drain_one()
                        pending_epi = (hs, den4, y2)
                    emit_epi(pending_epi)

                # ---- wo projection + residual (feature-major xmT) ----
                with tc.tile_pool(name="ps2b", bufs=1, space="PSUM") as ps2b:
                    for n4 in range(4):
                        att_ps = [ps2b.tile([128, TQ], f32, tag="att", bufs=8,
                                            name=f"att{n4}_{_k}") for _k in range(4)]
                        wo_sb = wsp.tile([128, NT, TQ], bf16, tag="wslab", bufs=2,
                                         name=f"wo_sb{n4}")
                        nc.sync.dma_start(
                            wo_sb[:],
                            wo[:, NT * TQ * n4:NT * TQ * (n4 + 1)].rearrange(
                                "p (g t) -> p g t", t=TQ))
                        for k in range(4):
                            for h in range(NH):
                                nc.tensor.matmul(att_ps[k][:],
                                                 lhsT=wo_sb[:, h, 128 * k:128 * (k + 1)],
                                                 rhs=yT[:, h],
                                                 start=(h == 0), stop=(h == NH - 1))
                            n = 4 * n4 + k
                            nc.vector.tensor_tensor(xmT[:, n], att_ps[k][:],
                                                    xin[:, n], Alu.add)
            # attn pool closed

            # ---- MLP ----
            with tc.tile_pool(name="mlp", bufs=1) as mp:
                h2T = mp.tile([128, NT, TQ], bf16, tag="h2T")
                a_sb = mp.tile([128, NF, TQ], bf16, tag="a_sb")

                with tc.tile_pool(name="ps3", bufs=1, space="PSUM") as ps3:
                    ssq2 = ps3.tile([1, TQ], f32, tag="row", bufs=2)
                    for i in range(NT):
                        xsq = wpool.tile([128, TQ], bf16, tag="xsq", bufs=6)
                        nc.vector.tensor_tensor(xsq[:], xmT[:, i], xmT[:, i],
                                                Alu.mult)
                        nc.tensor.matmul(ssq2[:], lhsT=ones[:], rhs=xsq[:],
                                         start=(i == 0), stop=(i == NT - 1))
                    s2rows = rsqrt_rows(ssq2, 1.0 / C, EPS, 1)
                    s2b = bcast_row(s2rows, 0)
                    for i in range(NT):
                        nc.vector.tensor_tensor(h2T[:, i], xmT[:, i], s2b[:],
                                                Alu.mult)

                # fc + relu^2 (feature-major a)
                with tc.tile_pool(name="ps3b", bufs=1, space="PSUM") as ps3b:
                    for jc in range(16):
                        f_ps = [ps3b.tile([128, TQ], f32, tag="mm", bufs=8,
                                          name=f"fps{jc}_{_k}") for _k in range(4)]
                        wfc_sb = wsp.tile([128, NT, TQ], bf16, tag="wslab", bufs=2,
                                          name=f"wfc_sb{jc}")
                        nc.sync.dma_start(
                            wfc_sb[:],
                            wfc[:, NT * TQ * jc:NT * TQ * (jc + 1)].rearrange(
                                "p (g t) -> p g t", t=TQ))
                        for jf in range(4):
                            for i in range(NT):
                                nc.tensor.matmul(
                                    f_ps[jf][:],
                                    lhsT=wfc_sb[:, i, 128 * jf:128 * (jf + 1)],
                                    rhs=h2T[:, i],
                                    start=(i == 0), stop=(i == NT - 1))
                            f = 4 * jc + jf
                            r_bf = wpool.tile([128, TQ], bf16, tag="r_bf")
                            nc.scalar.activation(r_bf[:], f_ps[jf][:], Act.Relu)
                            nc.vector.tensor_tensor(a_sb[:, f], r_bf[:], r_bf[:],
                                                    Alu.mult)

                # proj: weight-stationary, feature-major output + residual
                with tc.tile_pool(name="ps4", bufs=1, space="PSUM") as ps4:
                    for cg in range(4):
                        o_ps = [ps4.tile([128, TQ], f32, tag="o", bufs=8,
                                         name=f"ops{cg}_{_k}") for _k in range(4)]
                        for f8 in range(4):
                            wp_sb = wsp.tile([128, 16, TQ], bf16, tag="wslab",
                                             bufs=2, name=f"wp{cg}_{f8}")
                            base = (cg * 4 + f8) * 16 * TQ
                            nc.sync.dma_start(
                                wp_sb[:],
                                wproj[:, base:base + 16 * TQ].rearrange(
                                    "p (g t) -> p g t", t=TQ))
                            for fl in range(16):
                                f = 16 * f8 + fl
                                for cl in range(4):
                                    nc.tensor.matmul(
                                        o_ps[cl][:],
                                        lhsT=wp_sb[:, fl, 128 * cl:128 * (cl + 1)],
                                        rhs=a_sb[:, f],
                                        start=(f == 0), stop=(f == NF - 1))
                        for cl in range(4):
                            c = 4 * cg + cl
                            ov = wpool.tile([128, TQ], f32, tag="sbcast")
                            nc.vector.tensor_tensor(ov[:], o_ps[cl][:],
                                                    xmT[:, c], Alu.add)
                            nc.sync.dma_start(
                                out_fm[128 * c:128 * (c + 1), :], ov[:])

    nc.compile()
    return nc


def _core_perm(r):
    # zig-zag: front 256 tokens + mirrored back 256 tokens
    return np.concatenate([np.arange(256 * r, 256 * (r + 1)),
                           np.arange(T - 256 * (r + 1), T - 256 * r)])


def _make_in_maps(x, cos, sin, weights_b):
    import ml_dtypes
    bf = ml_dtypes.bfloat16
    cosT = cos[0, :, 0, :].T  # [64, T]
    sinT = sin[0, :, 0, :].T
    cscf = np.concatenate([cosT, cosT], axis=0)   # [128, T]
    cssf = np.concatenate([sinT, -sinT], axis=0)
    in_maps = []
    for c in range(NCORES):
        b, r = divmod(c, 4)
        perm = _core_perm(r)
        qf, qb = perm[:HQ], perm[HQ:]
        mf = np.arange(8 * 128)[:, None] <= qf[None, :]    # front: keys 0..1023
        mb = np.arange(16 * 128)[:, None] <= qb[None, :]   # back: keys 0..2047
        m = {
            "xT": np.ascontiguousarray(x[b, perm, :].T).astype(bf),
            "csc": np.ascontiguousarray(cscf[:, perm]).astype(bf),
            "css": np.ascontiguousarray(cssf[:, perm]).astype(bf),
            "mask": np.concatenate([mf, mb], axis=0).astype(bf),
        }
        m.update(weights_b)
        in_maps.append(m)
    return in_maps


def _prep_weights(wq, wk, wv, wo, w_fc, w_proj):
    import ml_dtypes
    bf = ml_dtypes.bfloat16

    def tile_w(w, chunk):
        # [R, F] -> [128, (F//chunk) * (R//128) * chunk]: per output column
        # chunk, row-blocks become contiguous along the free axis
        R, F = w.shape
        t = w.reshape(R // 128, 128, F // chunk, chunk)
        t = t.transpose(1, 2, 0, 3)  # [128, F//chunk, R//128, chunk]
        return np.ascontiguousarray(t.reshape(128, -1)).astype(bf)

    wproj_f = np.asarray(w_proj, np.float32)
    # proj layout: slabs of (cg, f8): [p, cg, f8, f_local 16, c_local 4, 128]
    t = wproj_f.reshape(4, 16, 128, 4, 4, 128)  # [f8, fl, p, cg, cl, col]
    t = t.transpose(2, 3, 0, 1, 4, 5)           # [p, cg, f8, fl, cl, col]
    wproj_t = np.ascontiguousarray(t.reshape(128, -1)).astype(bf)

    return {
        "wq": tile_w(np.asarray(wq, np.float32), TQ),
        "wk": tile_w(np.asarray(wk, np.float32), NKV * HD),
        "wv": tile_w(np.asarray(wv, np.float32), NKV * HD),
        "wo": tile_w(np.asarray(wo, np.float32), TQ),
        "wfc": tile_w(np.asarray(w_fc, np.float32), TQ),
        "wproj": wproj_t,
    }


def kernel(x, cos, sin, wq, wk, wv, wo, w_fc, w_proj):
    global _CACHE
    from concourse.bass_utils import run_bass_kernel_spmd

    x = np.asarray(x, np.float32)
    cos = np.asarray(cos, np.float32)
    sin = np.asarray(sin, np.float32)
    weights_b = _prep_weights(wq, wk, wv, wo, w_fc, w_proj)

    if _CACHE is None:
        _CACHE = _build()
    nc = _CACHE

    in_maps = _make_in_maps(x, cos, sin, weights_b)
    res = run_bass_kernel_spmd(nc, in_maps, list(range(NCORES)))
    out = np.empty((B, T, C), np.float32)
    for c in range(NCORES):
        b, r = divmod(c, 4)
        out[b, _core_perm(r), :] = res.results[c]["out"].T
    return out
